# revision 26
# baseline (speedup 1.0000x reference)
"""Trainium2 Bass kernel for nn_EncoderLayer_45423574122725.

Data-parallel over batch: 8 batch elements -> 8 NeuronCores, full pipeline
per core:
  radix-2 split-DFT rfft (fp32 matmuls + DVE twiddle combine) -> top-8 bins
  per (b,d) via DVE max8/match_replace -> masked-spectrum trig resynthesis
  (f16 matmuls) -> growth layer (matmul + first-diff + EMA via
  tensor_tensor_scan) -> layernorm -> sigmoid FF -> layernorm -> level layer
  (fused matmuls + EMA scan).

The FFT smoothing convs in the reference are exact exponential moving
averages (verified algebraically + numerically), implemented with the DVE
tensor_tensor_scan recurrence  state = a*state + b  along the free dim.
"""
import os
import sys
import types

sys.path.insert(0, "/opt/trn_rl_repo")

import numpy as np

import concourse.bacc as bacc
import concourse.bass as bass
import concourse.mybir as mybir
from concourse import tile

f32 = mybir.dt.float32
f16 = mybir.dt.float16
AL = mybir.AluOpType
AF = mybir.ActivationFunctionType
AX = mybir.AxisListType

T = 1024          # seq len
D = 512           # d_model
F = 511           # rfft bins 1..511 (LOW_FREQ=1, Nyquist excluded)
PRED = 256
C = 64            # level channels
FFN = 2048
EPS = 1e-5
NT = T // 128     # 8 time tiles
ND = D // 128     # 4 feature tiles
NF = FFN // 128   # 16 ffn tiles
KB = [0, 128, 256, 384, 511]   # bin-tile boundaries (bin = col+1)

_CACHE: dict = {}


def _ensure_axon_hooks():
    """Install the NTFF profile hook registry if the image's antenv lacks it."""
    try:
        from antenv.axon_hooks import get_axon_ntff_profile_hook  # noqa: F401
        return
    except ImportError:
        pass
    import antenv

    mod = types.ModuleType("antenv.axon_hooks")
    _h = [None]

    def _set(h):
        _h[0] = h

    def _get():
        return _h[0]

    mod.set_axon_ntff_profile_hook = _set
    mod.get_axon_ntff_profile_hook = _get
    sys.modules["antenv.axon_hooks"] = mod
    antenv.axon_hooks = mod
    try:
        from trn_agent_boot.trn_boot import _ntff_profile_via_ctypes
        _set(_ntff_profile_via_ctypes("/opt/axon/libaxon_pjrt.so"))
    except Exception:
        pass


def _build_program(flags):
    """Emit the single-core Bass/Tile program (SPMD across 8 cores).

    flags: (has_gob, has_bu, has_ffb) — whether those bias terms are nonzero.
    """
    has_gob, has_bu, has_ffb = flags
    from concourse import tile_utils
    tile_utils.max_sbuf_usage = 208 * 1024  # cayman usable; default cap is stale

    nc = bacc.Bacc("TRN2", target_bir_lowering=False, debug=False)

    # ---------------- DRAM I/O ----------------
    d_res = nc.dram_tensor("res", [T, D], f32, kind="ExternalInput")
    d_level = nc.dram_tensor("level", [T, C], f32, kind="ExternalInput")
    d_cs512 = nc.dram_tensor("cs512", [512, 514], f32, kind="ExternalInput")
    d_os512 = nc.dram_tensor("os512", [512, 514], f32, kind="ExternalInput")
    d_cs = nc.dram_tensor("cs", [F, T], f16, kind="ExternalInput")        # cos * 2/T
    d_snn = nc.dram_tensor("snn", [F, T], f16, kind="ExternalInput")      # +-sin * 2/T
    d_glinT = nc.dram_tensor("glinT", [D, D], f16, kind="ExternalInput")  # gl_in_w.T
    d_gloutT = nc.dram_tensor("gloutT", [D, D], f16, kind="ExternalInput")
    d_ffw1T = nc.dram_tensor("ffw1T", [D, FFN], f16, kind="ExternalInput")
    d_ffw2T = nc.dram_tensor("ffw2T", [FFN // 2, 2 * D], f16, kind="ExternalInput")
    d_wgs = nc.dram_tensor("wgs", [D, C], f16, kind="ExternalInput")
    d_lsw2 = nc.dram_tensor("lsw2", [D, C], f16, kind="ExternalInput")
    d_id = nc.dram_tensor("idm", [128, 128], f32, kind="ExternalInput")
    d_id16 = nc.dram_tensor("idm16", [128, 128], f16, kind="ExternalInput")
    d_chc = nc.dram_tensor("chc", [D, 4], f32, kind="ExternalInput")      # [a,1-a,v0,z0b]
    d_ccc = nc.dram_tensor("ccc", [C, 3], f32, kind="ExternalInput")      # [a,1-a,v0]
    d_g0 = nc.dram_tensor("g0", [1, D], f32, kind="ExternalInput")        # growth row 0
    if has_gob:
        d_ones = nc.dram_tensor("onesr", [1, D], f16, kind="ExternalInput")
        d_gob = nc.dram_tensor("gob", [1, D], f16, kind="ExternalInput")
    if has_bu:
        d_ones2 = nc.dram_tensor("onesr2", [1, D], f16, kind="ExternalInput")
        d_bu = nc.dram_tensor("bu", [1, C], f16, kind="ExternalInput")
    if has_ffb:
        d_fb = nc.dram_tensor("fb", [FFN, 1], f32, kind="ExternalInput")

    d_out_res = nc.dram_tensor("out_res", [T, D], f32, kind="ExternalOutput")
    d_out_level = nc.dram_tensor("out_level", [T, C], f32, kind="ExternalOutput")
    d_out_growth = nc.dram_tensor("out_growth", [T + 1, D], f32, kind="ExternalOutput")
    d_out_season = nc.dram_tensor("out_season", [T + PRED, D], f32, kind="ExternalOutput")

    from contextlib import ExitStack
    with tile.TileContext(nc) as tc, ExitStack() as _es:
        cp = _es.enter_context(tc.tile_pool(name="cp", bufs=1))
        sp = _es.enter_context(tc.tile_pool(name="sp", bufs=6))
        so = _es.enter_context(tc.tile_pool(name="so", bufs=2))
        pp = _es.enter_context(tc.tile_pool(name="pp", bufs=6, space="PSUM"))
        pq = _es.enter_context(tc.tile_pool(name="pq", bufs=2, space="PSUM"))

        dma = nc.sync.dma_start

        eps_col = cp.tile([128, 1], f32, tag="eps")
        nc.vector.memset(eps_col[:], EPS)

        # ======== stage-0 DMAs (front-of-queue: what the PE needs first) ====
        # even/odd rows of res into the slots later reused by sigmoid tiles
        d_res_eo = d_res.rearrange("(a two) d -> a two d", two=2)
        xe_sb, xo_sb = [], []
        for j in range(4):
            t_ = cp.tile([128, D], f32, tag=f"sg{j}")
            dma(out=t_[:], in_=d_res_eo[j * 128:(j + 1) * 128, 0, :])
            xe_sb.append(t_)
        for j in range(4):
            t_ = cp.tile([128, D], f32, tag=f"sg{4 + j}")
            dma(out=t_[:], in_=d_res_eo[j * 128:(j + 1) * 128, 1, :])
            xo_sb.append(t_)

        c512_sb, s512_sb = [], []
        for kk in range(4):
            t_ = cp.tile([128, 514], f32, tag=f"e5{kk}")
            dma(out=t_[:], in_=d_cs512[kk * 128:(kk + 1) * 128, :])
            c512_sb.append(t_[:, 0:257])
            s512_sb.append(t_[:, 257:514])
        oc_sb, os_sb = [], []
        for kk in range(4):
            t_ = cp.tile([128, 514], f32, tag=f"o5{kk}")
            dma(out=t_[:], in_=d_os512[kk * 128:(kk + 1) * 128, :])
            oc_sb.append(t_[:, 0:257])
            os_sb.append(t_[:, 257:514])

        def bcast_row(dram, tag, n):
            t_ = cp.tile([128, n], f32, tag=tag)
            dma(out=t_[:], in_=dram[0:1, :].broadcast_to((128, n)))
            return t_

        id_sb = cp.tile([128, 128], f32, tag="id")
        dma(out=id_sb[:], in_=d_id[:])
        id16_sb = cp.tile([128, 128], f16, tag="id16")
        dma(out=id16_sb[:], in_=d_id16[:])

        # remaining inputs (ordered roughly by first use); issue on the
        # gpsimd queue so the sync queue stays clear for the critical path,
        # and gate them behind the first E/O matmul chain so the rfft inputs
        # get full DMA bandwidth at kernel start
        _gated = []

        def gdma(out, in_):
            bi = nc.gpsimd.dma_start(out=out, in_=in_)
            _gated.append(bi)
            return bi
        res_sb = []
        for j in range(NT):
            t_ = cp.tile([128, D], f32, tag=f"res{j}")
            gdma(out=t_[:], in_=d_res[j * 128:(j + 1) * 128, :])
            res_sb.append(t_)

        cs_sb, snn_sb = [], []
        for i in range(4):
            kw = KB[i + 1] - KB[i]
            t_ = cp.tile([128, T], f16, tag=f"cs{i}")
            gdma(out=t_[0:kw, :], in_=d_cs[KB[i]:KB[i + 1], :])
            cs_sb.append(t_)
        for i in range(4):
            kw = KB[i + 1] - KB[i]
            t_ = cp.tile([128, T], f16, tag=f"sn{i}")
            gdma(out=t_[0:kw, :], in_=d_snn[KB[i]:KB[i + 1], :])
            snn_sb.append(t_)

        glinT_sb, gloutT_sb = [], []
        for i in range(ND):
            t_ = cp.tile([128, D], f16, tag=f"gin{i}")
            gdma(out=t_[:], in_=d_glinT[i * 128:(i + 1) * 128, :])
            glinT_sb.append(t_)
        for i in range(ND):
            t_ = cp.tile([128, D], f16, tag=f"got{i}")
            gdma(out=t_[:], in_=d_gloutT[i * 128:(i + 1) * 128, :])
            gloutT_sb.append(t_)

        wgs_sb, lsw2_sb = [], []
        for i in range(ND):
            t_ = cp.tile([128, C], f16, tag=f"wgs{i}")
            gdma(out=t_[:], in_=d_wgs[i * 128:(i + 1) * 128, :])
            wgs_sb.append(t_)
        for i in range(ND):
            t_ = cp.tile([128, C], f16, tag=f"lsw{i}")
            gdma(out=t_[:], in_=d_lsw2[i * 128:(i + 1) * 128, :])
            lsw2_sb.append(t_)

        chc_sb = []   # per ch-tile: cols [alpha, 1-alpha, v0, z0b]
        for m in range(ND):
            t_ = cp.tile([128, 4], f32, tag=f"chc{m}")
            gdma(out=t_[:], in_=d_chc[m * 128:(m + 1) * 128, :])
            chc_sb.append(t_)
        ccc_sb = cp.tile([C, 3], f32, tag="ccc")
        gdma(out=ccc_sb[:], in_=d_ccc[:])

        lvl_sb = []
        for j in range(NT):
            t_ = cp.tile([128, C], f32, tag=f"lvl{j}")
            gdma(out=t_[:], in_=d_level[j * 128:(j + 1) * 128, :])
            lvl_sb.append(t_)

        if has_gob:
            ones_sb = cp.tile([1, D], f16, tag="ones")
            dma(out=ones_sb[:], in_=d_ones[:])
            gob_sb = cp.tile([1, D], f16, tag="gob")
            dma(out=gob_sb[:], in_=d_gob[:])
        if has_bu:
            ones2_sb = cp.tile([1, D], f16, tag="ones2")
            dma(out=ones2_sb[:], in_=d_ones2[:])
            bu_sb = cp.tile([1, C], f16, tag="bu")
            dma(out=bu_sb[:], in_=d_bu[:])
        if has_ffb:
            fb_sb = cp.tile([128, NF], f32, tag="fb")
            for fi in range(NF):
                dma(out=fb_sb[:, fi:fi + 1], in_=d_fb[fi * 128:(fi + 1) * 128, :])

        # growth row 0 is input-independent (v0 @ W + b): DMA straight through
        dma(out=d_out_growth[0:1, :], in_=d_g0[:])

        # level input transpose (PE filler while the rfft waits on DMA)
        lvT = cp.tile([64, T], f16, tag="lvT")
        for j in range(NT):
            ps = pq.tile([128, 512], f32, tag="s")
            nc.tensor.transpose(ps[0:C, 0:128], lvl_sb[j][:, 0:C], id_sb[:])
            nc.scalar.copy(lvT[:, j * 128:(j + 1) * 128], ps[0:C, 0:128])

        # ======= S1: split-DFT (E/O bins 0..256, fp32) + twiddle combine ====
        # ======= S2: top-8 mask per d -> MR/MI [d, k] (f16) =================
        mr_sb, mi_sb = [], []
        for i in range(ND):
            psER = pp.tile([128, 512], f32, tag="m")
            psEI = pp.tile([128, 512], f32, tag="m")
            psOR = pp.tile([128, 512], f32, tag="m")
            psOI = pp.tile([128, 512], f32, tag="m")
            for (ps, src, tab) in ((psER, xe_sb, c512_sb), (psEI, xe_sb, s512_sb),
                                   (psOR, xo_sb, oc_sb), (psOI, xo_sb, os_sb)):
                for kk in range(4):
                    bi = nc.tensor.matmul(
                        ps[:, 0:257], src[kk][:, i * 128:(i + 1) * 128],
                        tab[kk], start=(kk == 0), stop=(kk == 3))
            if i == 0 and _gated:
                from concourse.tile_rust import add_dep_helper
                for g in _gated:
                    add_dep_helper(bi.ins, g.ins,
                                   reason="bulk DMA yields to rfft inputs")
                _gated.clear()

            xr = cp.tile([128, 512], f32, tag="xr0")
            xi = cp.tile([128, 512], f32, tag="xi0")
            amp = cp.tile([128, 512], f32, tag=f"amp{i % 2}")
            rep = cp.tile([128, 512], f32, tag=f"rep{i % 2}")
            TT = nc.vector.tensor_tensor
            # odd tables carry the twiddle; E mirrors by conjugate symmetry.
            # lo bins 1..256 -> cols 0..255; hi bins (reversed) store the
            # NEGATED imag part; snn rows 256+ are negated on host to match.
            # (only one PSUM operand allowed per DVE op: evac the odd pair)
            nc.scalar.copy(amp[:, 0:257], psOR[:, 0:257])
            nc.scalar.copy(rep[:, 0:257], psOI[:, 0:257])
            TT(xr[:, 0:256], psER[:, 1:257], amp[:, 1:257], AL.add)
            TT(xr[:, 256:511], psER[:, 255:0:-1], amp[:, 255:0:-1], AL.subtract)
            TT(xi[:, 0:256], psEI[:, 1:257], rep[:, 1:257], AL.add)
            TT(xi[:, 256:511], psEI[:, 255:0:-1], rep[:, 255:0:-1], AL.subtract)

            # amplitude^2 and top-8 mask
            nc.scalar.activation(amp[:, 0:F], xr[:, 0:F], AF.Square)
            nc.scalar.activation(rep[:, 0:F], xi[:, 0:F], AF.Square)
            TT(amp[:, 0:F], amp[:, 0:F], rep[:, 0:F], AL.add)
            mx8 = sp.tile([128, 8], f32, tag="mx8")
            nc.vector.max(mx8[:], amp[:, 0:F])
            # top-8 selection as a threshold on the 8th-largest amplitude,
            # fused into the masking multiplies
            mr = cp.tile([128, 512], f16, tag=f"mr{i}")
            mi = cp.tile([128, 512], f16, tag=f"mi{i}")
            nc.vector.scalar_tensor_tensor(mr[:, 0:F], amp[:, 0:F],
                                           mx8[:, 7:8], xr[:, 0:F],
                                           AL.is_ge, AL.mult)
            nc.vector.scalar_tensor_tensor(mi[:, 0:F], amp[:, 0:F],
                                           mx8[:, 7:8], xi[:, 0:F],
                                           AL.is_ge, AL.mult)
            mr_sb.append(mr)
            mi_sb.append(mi)

        # ======= S3: MRt/MIt [k,d] (f16) and MRW/MIW [k,c] (f16) =========
        mrt_sb, mit_sb = [], []
        for kk in range(4):
            kw = KB[kk + 1] - KB[kk]
            for (src, dstl, tg) in ((mr_sb, mrt_sb, "mrt"), (mi_sb, mit_sb, "mit")):
                ps = pp.tile([128, 512], f32, tag="m")
                for i in range(ND):
                    nc.tensor.matmul(
                        ps[0:kw, i * 128:(i + 1) * 128],
                        src[i][:, KB[kk]:KB[kk + 1]], id16_sb[:],
                        start=True, stop=True)
                t_ = cp.tile([128, 512], f16, tag=f"{tg}{kk}")
                nc.scalar.copy(t_[0:kw, :], ps[0:kw, :])
                dstl.append(t_)

        mrw_sb, miw_sb = [], []
        for kk in range(4):
            kw = KB[kk + 1] - KB[kk]
            for (src, dstl, tg) in ((mr_sb, mrw_sb, "mrw"), (mi_sb, miw_sb, "miw")):
                ps = pq.tile([128, 512], f32, tag="s")
                for i in range(ND):
                    nc.tensor.matmul(
                        ps[0:kw, 0:C], src[i][:, KB[kk]:KB[kk + 1]],
                        lsw2_sb[i][:], start=(i == 0), stop=(i == ND - 1))
                t_ = cp.tile([128, C], f16, tag=f"{tg}{kk}")
                nc.scalar.copy(t_[0:kw, :], ps[0:kw, 0:C])
                dstl.append(t_)

        # ======= S4: season [tau,d]; res2 = res - season; season out =====
        sea_sb, res2_sb = [], []
        for j in range(NT):
            ps = pp.tile([128, 512], f32, tag="m")
            for kk in range(4):
                kw = KB[kk + 1] - KB[kk]
                nc.tensor.matmul(
                    ps[:], cs_sb[kk][0:kw, j * 128:(j + 1) * 128],
                    mrt_sb[kk][0:kw, 0:D], start=(kk == 0), stop=False)
                nc.tensor.matmul(
                    ps[:], snn_sb[kk][0:kw, j * 128:(j + 1) * 128],
                    mit_sb[kk][0:kw, 0:D], start=False, stop=(kk == 3))
            sea = cp.tile([128, D], f32, tag=f"sea{j % 4}")
            nc.scalar.copy(sea[:], ps[:])
            r2 = cp.tile([128, D], f32, tag=f"r2{j}")
            nc.vector.tensor_tensor(r2[:], res_sb[j][:], ps[:], AL.subtract)
            sea_sb.append(sea)
            res2_sb.append(r2)
            dma(out=d_out_season[j * 128:(j + 1) * 128, :], in_=sea[:])
            if j < 2:   # periodic extension: rows 1024..1279 = rows 0..255
                dma(out=d_out_season[T + j * 128:T + (j + 1) * 128, :], in_=sea[:])

        # ======= S5: res2T [d,t] (f16) ===================================
        res2T_sb = []
        for i in range(ND):
            t_ = cp.tile([128, T], f16, tag=f"r2t{i}")
            for jh in range(2):
                ps = pp.tile([128, 512], f32, tag="m")
                for j4 in range(4):
                    j = jh * 4 + j4
                    nc.tensor.transpose(
                        ps[:, j4 * 128:(j4 + 1) * 128],
                        res2_sb[j][:, i * 128:(i + 1) * 128], id_sb[:])
                nc.scalar.copy(t_[:, jh * 512:(jh + 1) * 512], ps[:])
            res2T_sb.append(t_)

        # ======= S6: vT [ch,t] = glinT.T @ res2T  (into vd slots) ========
        vT_sb = []
        for m in range(ND):
            t_ = cp.tile([128, T], f32, tag=f"vd{m}")
            for th in range(2):
                ps = pp.tile([128, 512], f32, tag="m")
                for i in range(ND):
                    nc.tensor.matmul(
                        ps[:], glinT_sb[i][:, m * 128:(m + 1) * 128],
                        res2T_sb[i][:, th * 512:(th + 1) * 512],
                        start=(i == 0), stop=(i == ND - 1))
                nc.scalar.copy(t_[:, th * 512:(th + 1) * 512], ps[:])
            vT_sb.append(t_)

        # ======= S7: vdiff, u=(1-a)*vd, EMA scan -> sT [ch, 1+t] (f16) ===
        sT_sb = []
        for m in range(ND):
            vd = cp.tile([128, T], f32, tag=f"r2t{m}")
            nc.vector.tensor_tensor(vd[:, 1:T], vT_sb[m][:, 1:T],
                                    vT_sb[m][:, 0:T - 1], AL.subtract)
            nc.vector.tensor_tensor(vd[:, 0:1], vT_sb[m][:, 0:1],
                                    chc_sb[m][:, 3:4], AL.subtract)
            u = vT_sb[m]   # overwrite vT slot elementwise from vd
            nc.vector.tensor_scalar(u[:], vd[:], chc_sb[m][:, 1:2], None, AL.mult)
            st = cp.tile([128, 1056], f16, tag=f"st{m}")
            nc.vector.tensor_copy(st[:, 0:1], chc_sb[m][:, 2:3])
            nc.vector.tensor_tensor_scan(
                st[:, 1:T + 1], chc_sb[m][:, 0:1].broadcast_to((128, T)), u[:],
                chc_sb[m][:, 2:3], AL.mult, AL.add)
            sT_sb.append(st)

        # ======= level path: u-accum [c,t], scan, out ====================
        usb = cp.tile([64, T], f32, tag="r2t0")
        lvs = cp.tile([64, T], f32, tag="r2t1")
        for th in range(2):
            ps = pq.tile([128, 512], f32, tag="s")
            for kk in range(4):
                kw = KB[kk + 1] - KB[kk]
                nc.tensor.matmul(ps[0:C, :], mrw_sb[kk][0:kw, :],
                                 cs_sb[kk][0:kw, th * 512:(th + 1) * 512],
                                 start=(kk == 0), stop=False)
                nc.tensor.matmul(ps[0:C, :], miw_sb[kk][0:kw, :],
                                 snn_sb[kk][0:kw, th * 512:(th + 1) * 512],
                                 start=False, stop=False)
            for m in range(ND):
                nc.tensor.matmul(ps[0:C, :], wgs_sb[m][:],
                                 sT_sb[m][:, th * 512:(th + 1) * 512],
                                 start=False, stop=(m == ND - 1 and not has_bu))
            if has_bu:
                nc.tensor.matmul(ps[0:C, :], bu_sb[:], ones2_sb[:],
                                 start=False, stop=True)
            nc.vector.scalar_tensor_tensor(
                usb[:, th * 512:(th + 1) * 512], lvT[:, th * 512:(th + 1) * 512],
                ccc_sb[:, 1:2], ps[0:C, :], AL.mult, AL.add)
        nc.vector.tensor_tensor_scan(
            lvs[:], ccc_sb[:, 0:1].broadcast_to((64, T)), usb[:],
            ccc_sb[:, 2:3], AL.mult, AL.add)
        for j in range(NT):
            ps = pq.tile([128, 512], f32, tag="s")
            nc.tensor.transpose(ps[:, 0:C], lvs[:, j * 128:(j + 1) * 128],
                                id_sb[0:64, 0:64])
            lo = so.tile([128, C], f32, tag="lvo")
            nc.scalar.copy(lo[:], ps[:, 0:C])
            dma(out=d_out_level[j * 128:(j + 1) * 128, :], in_=lo[:])

        # FF weights arrive into slots freed by the level/synthesis stages
        ffw1_sb = []
        for i in range(ND):
            for h in range(2):
                t_ = cp.tile([128, T], f16, tag=(f"cs{i}" if h == 0 else f"sn{i}"))
                gdma(out=t_[:], in_=d_ffw1T[i * 128:(i + 1) * 128,
                                           h * 1024:(h + 1) * 1024])
                ffw1_sb.append(t_)  # index 2*i + h

        ffw2_sb = []
        for f in range(8):
            t_ = cp.tile([128, 1024], f16, tag=f"ff2{f}")
            gdma(out=t_[:], in_=d_ffw2T[f * 128:(f + 1) * 128, :])
            ffw2_sb.append(t_)

        # ======= S8: growth rows 1..1024; x1 = res2 - growth[1:] =========
        x1_sb = []
        for j in range(NT):
            ps = pp.tile([128, 512], f32, tag="m")
            for m in range(ND):
                nc.tensor.matmul(
                    ps[:], sT_sb[m][:, j * 128 + 1:(j + 1) * 128 + 1],
                    gloutT_sb[m][:], start=(m == 0),
                    stop=(m == ND - 1 and not has_gob))
            if has_gob:
                nc.tensor.matmul(ps[:], ones_sb[0:1, 0:128], gob_sb[:],
                                 start=False, stop=True)
            x1 = cp.tile([128, D], f32, tag=f"sea{j % 4}")
            nc.vector.tensor_tensor(x1[:], res2_sb[j][:], ps[:], AL.subtract)
            gr = cp.tile([128, D], f32, tag=f"r2{j}")
            nc.scalar.copy(gr[:], ps[:])
            dma(out=d_out_growth[j * 128 + 1:(j + 1) * 128 + 1, :], in_=gr[:])
            x1_sb.append(x1)

        # ======= layer norm: z = (x - mean) * rstd (gamma/beta folded) ===
        def norm_z(x_in, out_tile, j):
            st6 = sp.tile([128, 6], f32, tag="st6")
            nc.vector.bn_stats(st6[:], x_in[:])
            mv = sp.tile([128, 2], f32, tag="mv")
            nc.vector.bn_aggr(mv[:], st6[:])
            std = sp.tile([128, 1], f32, tag="col")
            nc.scalar.activation(std[:], mv[:, 1:2], AF.Sqrt, bias=eps_col[:])
            rstd = sp.tile([128, 1], f32, tag="col")
            nc.vector.reciprocal(rstd[:], std[:])
            nc.vector.tensor_scalar(out_tile[:], x_in[:], mv[:, 0:1], rstd[:],
                                    AL.subtract, AL.mult)
            return out_tile

        # ======= S9: norm1 -> z1 [t,d] (gamma1 folded into ffw1) =========
        res3_sb = []
        for j in range(NT):
            out = cp.tile([128, D], f32, tag=f"res{j}")
            norm_z(x1_sb[j], out, j)
            res3_sb.append(out)

        # ======= S10: res3T [d,t] (f16) ==================================
        res3T_sb = []
        for i in range(ND):
            t_ = cp.tile([128, T], f16, tag=f"vd{i}")
            for jh in range(2):
                ps = pp.tile([128, 512], f32, tag="m")
                for j4 in range(4):
                    j = jh * 4 + j4
                    nc.tensor.transpose(
                        ps[:, j4 * 128:(j4 + 1) * 128],
                        res3_sb[j][:, i * 128:(i + 1) * 128], id_sb[:])
                nc.scalar.copy(t_[:, jh * 512:(jh + 1) * 512], ps[:])
            res3T_sb.append(t_)

        # ======= S11: FF1 (gamma1-scaled weights) + sigmoid(+bias) =======
        sig_sb = []
        for fi in range(NF):
            h, fo = fi // 8, fi % 8
            sg = cp.tile([128, T], f16, tag=f"sg{fi}")
            for th in range(2):
                ps = pp.tile([128, 512], f32, tag="m")
                for i in range(ND):
                    nc.tensor.matmul(
                        ps[:], ffw1_sb[2 * i + h][:, fo * 128:(fo + 1) * 128],
                        res3T_sb[i][:, th * 512:(th + 1) * 512],
                        start=(i == 0), stop=(i == ND - 1))
                if has_ffb:
                    nc.scalar.activation(sg[:, th * 512:(th + 1) * 512], ps[:],
                                         AF.Sigmoid, bias=fb_sb[:, fi:fi + 1])
                else:
                    nc.scalar.activation(sg[:, th * 512:(th + 1) * 512], ps[:],
                                         AF.Sigmoid)
            sig_sb.append(sg)

        # ======= S12/S13: FF2 + residual + norm2 -> out ==================
        for j in range(NT):
            ps = pp.tile([128, 512], f32, tag="m")
            for fi in range(NF):
                nc.tensor.matmul(
                    ps[:], sig_sb[fi][:, j * 128:(j + 1) * 128],
                    ffw2_sb[fi % 8][:, (fi // 8) * 512:(fi // 8 + 1) * 512],
                    start=(fi == 0), stop=(fi == NF - 1))
            u2 = cp.tile([128, D], f32, tag=f"sea{j % 4}")
            nc.vector.tensor_tensor(u2[:], res3_sb[j][:], ps[:], AL.add)
            out = cp.tile([128, D], f32, tag=f"st{j % 4}")
            norm_z(u2, out, j)
            dma(out=d_out_res[j * 128:(j + 1) * 128, :], in_=out[:])

    nc.compile()
    return nc


def _host_prep(inputs):
    """Build per-core input maps (numpy only)."""
    def sig(x):
        return 1.0 / (1.0 + np.exp(-x.astype(np.float64)))

    res = np.ascontiguousarray(inputs["res"], dtype=np.float32)
    level = np.ascontiguousarray(inputs["level"], dtype=np.float32)

    tp = np.arange(512)
    k2 = np.arange(257)
    ang_e = 2.0 * np.pi * np.outer(2 * tp, k2) / T
    ang_o = 2.0 * np.pi * np.outer(2 * tp + 1, k2) / T
    cs512 = np.concatenate(
        [np.cos(ang_e), -np.sin(ang_e)], axis=1).astype(np.float32)  # (512, 514)
    os512 = np.concatenate(
        [np.cos(ang_o), -np.sin(ang_o)], axis=1).astype(np.float32)

    t = np.arange(T)
    k = np.arange(1, F + 1)
    ang_kt = 2.0 * np.pi * np.outer(k, t) / T
    cs = (np.cos(ang_kt) * (2.0 / T)).astype(np.float16)
    snn_f = -np.sin(ang_kt) * (2.0 / T)
    snn_f[256:] = -snn_f[256:]   # hi bins store negated imag part on device
    snn = snn_f.astype(np.float16)

    gl_in_w = inputs["gl_in_w"].astype(np.float64)
    gl_out_w = inputs["gl_out_w"].astype(np.float64)
    alpha_ch = np.repeat(sig(inputs["gl_sw"]).reshape(-1), 64)      # (512,)
    v0_ch = inputs["gl_v0"].reshape(-1).astype(np.float64)
    z0b = (inputs["gl_z0"].reshape(-1).astype(np.float64)
           - inputs["gl_in_b"].astype(np.float64))

    alpha_c = sig(inputs["ll_sw"]).reshape(-1)                      # (64,)
    ll_gw = inputs["ll_gw"].astype(np.float64)
    ll_sw2 = inputs["ll_sw2"].astype(np.float64)
    b_g = (inputs["gl_out_b"].astype(np.float64) @ ll_gw.T
           + inputs["ll_gb"].astype(np.float64))
    wgs = (gl_out_w.T @ ll_gw.T) * alpha_c[None, :]
    lsw2 = ll_sw2.T * (-(1.0 - alpha_c))[None, :]
    bias_u = (-(1.0 - alpha_c) * inputs["ll_sb"].astype(np.float64)
              + alpha_c * b_g)

    chc = np.stack([alpha_ch, 1.0 - alpha_ch, v0_ch, z0b], axis=1)
    ccc = np.stack([alpha_c, 1.0 - alpha_c,
                    inputs["ll_v0"].reshape(-1).astype(np.float64)], axis=1)
    g0 = v0_ch @ gl_out_w.T + inputs["gl_out_b"].astype(np.float64)

    # layernorm gamma/beta folding: norm1's gamma is absorbed into ff_w1 and
    # its beta into the sigmoid bias. The residual/norm2 gamma/beta must be
    # identity for this build (true for the reference model: gamma=1, beta=0).
    n1g = inputs["n1_g"].astype(np.float64)
    n1b = inputs["n1_b"].astype(np.float64)
    assert np.all(n1g == 1.0) and np.all(inputs["n2_g"] == 1.0), \
        "non-identity layernorm gamma not supported by this build"
    assert np.all(n1b == 0.0) and np.all(inputs["n2_b"] == 0.0), \
        "nonzero layernorm beta not supported by this build"
    ffw1 = inputs["ff_w1"].astype(np.float64) * n1g[None, :]
    ffb = ffw1 @ n1b   # (FFN,) sigmoid bias

    has_gob = bool(np.any(inputs["gl_out_b"] != 0))
    has_bu = bool(np.any(bias_u != 0))
    has_ffb = bool(np.any(ffb != 0))

    shared = {
        "cs512": cs512, "os512": os512, "cs": cs, "snn": snn,
        "glinT": np.ascontiguousarray(gl_in_w.T, dtype=np.float16),
        "gloutT": np.ascontiguousarray(gl_out_w.T, dtype=np.float16),
        "ffw1T": np.ascontiguousarray(ffw1.T, dtype=np.float16),
                "ffw2T": np.ascontiguousarray(
            np.concatenate([inputs["ff_w2"].T[:FFN // 2],
                            inputs["ff_w2"].T[FFN // 2:]], axis=1),
            dtype=np.float16),
        "wgs": wgs.astype(np.float16),
        "lsw2": lsw2.astype(np.float16),
        "idm": np.eye(128, dtype=np.float32),
        "idm16": np.eye(128, dtype=np.float16),
        "chc": chc.astype(np.float32),
        "ccc": ccc.astype(np.float32),
        "g0": g0.astype(np.float32).reshape(1, D),
    }
    if has_gob:
        shared["onesr"] = np.ones((1, D), dtype=np.float16)
        shared["gob"] = inputs["gl_out_b"].astype(np.float16).reshape(1, D)
    if has_bu:
        shared["onesr2"] = np.ones((1, D), dtype=np.float16)
        shared["bu"] = bias_u.astype(np.float16).reshape(1, C)
    if has_ffb:
        shared["fb"] = ffb.astype(np.float32).reshape(FFN, 1)

    in_maps = []
    for b in range(res.shape[0]):
        m = dict(shared)
        m["res"] = res[b]
        m["level"] = level[b]
        in_maps.append(m)
    return in_maps, (has_gob, has_bu, has_ffb)


def kernel(**inputs):
    _ensure_axon_hooks()
    from concourse.bass_utils import run_bass_kernel_spmd

    in_maps, flags = _host_prep(inputs)
    key = ("nc", flags)
    if key not in _CACHE:
        _CACHE[key] = _build_program(flags)
    nc = _CACHE[key]

    n = len(in_maps)
    kw = {}
    if os.environ.get("KERNEL_TRACE"):
        kw = dict(trace=True, tmpdir=os.environ.get("KERNEL_TRACE_DIR") or None)
    r_ = None
    for attempt in range(3):
        try:
            r_ = run_bass_kernel_spmd(nc, in_maps, list(range(n)), **kw)
            break
        except Exception:
            if attempt == 2:
                raise
            import time
            time.sleep(2.0)
    _CACHE["last_exec_time_ns"] = r_.exec_time_ns

    res_out = np.stack([r_.results[b]["out_res"] for b in range(n)])
    level_out = np.stack([r_.results[b]["out_level"] for b in range(n)])
    growth_out = np.stack([r_.results[b]["out_growth"] for b in range(n)])
    season_out = np.stack([r_.results[b]["out_season"] for b in range(n)])
    return (res_out.astype(np.float32), level_out.astype(np.float32),
            growth_out.astype(np.float32), season_out.astype(np.float32))


# revision 27
# speedup vs baseline: 1.0637x; 1.0637x over previous
"""Trainium2 Bass kernel for nn_EncoderLayer_45423574122725.

Data-parallel over batch: 8 batch elements -> 8 NeuronCores, full pipeline
per core:
  radix-2 split-DFT rfft (fp32 matmuls + DVE twiddle combine) -> top-8 bins
  per (b,d) via DVE max8/match_replace -> masked-spectrum trig resynthesis
  (f16 matmuls) -> growth layer (matmul + first-diff + EMA via
  tensor_tensor_scan) -> layernorm -> sigmoid FF -> layernorm -> level layer
  (fused matmuls + EMA scan).

The FFT smoothing convs in the reference are exact exponential moving
averages (verified algebraically + numerically), implemented with the DVE
tensor_tensor_scan recurrence  state = a*state + b  along the free dim.
"""
import os
import sys
import types

sys.path.insert(0, "/opt/trn_rl_repo")

import numpy as np

import concourse.bacc as bacc
import concourse.bass as bass
import concourse.mybir as mybir
from concourse import tile

f32 = mybir.dt.float32
f16 = mybir.dt.float16
AL = mybir.AluOpType
AF = mybir.ActivationFunctionType
AX = mybir.AxisListType

T = 1024          # seq len
D = 512           # d_model
F = 511           # rfft bins 1..511 (LOW_FREQ=1, Nyquist excluded)
PRED = 256
C = 64            # level channels
FFN = 2048
EPS = 1e-5
NT = T // 128     # 8 time tiles
ND = D // 128     # 4 feature tiles
NF = FFN // 128   # 16 ffn tiles
KB = [0, 128, 256, 384, 511]   # bin-tile boundaries (bin = col+1)

_CACHE: dict = {}


def _ensure_axon_hooks():
    """Install the NTFF profile hook registry if the image's antenv lacks it."""
    try:
        from antenv.axon_hooks import get_axon_ntff_profile_hook  # noqa: F401
        return
    except ImportError:
        pass
    import antenv

    mod = types.ModuleType("antenv.axon_hooks")
    _h = [None]

    def _set(h):
        _h[0] = h

    def _get():
        return _h[0]

    mod.set_axon_ntff_profile_hook = _set
    mod.get_axon_ntff_profile_hook = _get
    sys.modules["antenv.axon_hooks"] = mod
    antenv.axon_hooks = mod
    try:
        from trn_agent_boot.trn_boot import _ntff_profile_via_ctypes
        _set(_ntff_profile_via_ctypes("/opt/axon/libaxon_pjrt.so"))
    except Exception:
        pass


def _build_program(flags):
    """Emit the single-core Bass/Tile program (SPMD across 8 cores).

    flags: (has_gob, has_bu, has_ffb) — whether those bias terms are nonzero.
    """
    has_gob, has_bu, has_ffb = flags
    from concourse import tile_utils
    tile_utils.max_sbuf_usage = 208 * 1024  # cayman usable; default cap is stale

    nc = bacc.Bacc("TRN2", target_bir_lowering=False, debug=False)

    # ---------------- DRAM I/O ----------------
    d_res = nc.dram_tensor("res", [T, D], f32, kind="ExternalInput")
    d_level = nc.dram_tensor("level", [T, C], f32, kind="ExternalInput")
    d_cs512 = nc.dram_tensor("cs512", [512, 514], f32, kind="ExternalInput")
    d_os512 = nc.dram_tensor("os512", [512, 514], f32, kind="ExternalInput")
    d_cs = nc.dram_tensor("cs", [F, T], f16, kind="ExternalInput")        # cos * 2/T
    d_snn = nc.dram_tensor("snn", [F, T], f16, kind="ExternalInput")      # +-sin * 2/T
    d_glinT = nc.dram_tensor("glinT", [D, D], f16, kind="ExternalInput")  # gl_in_w.T
    d_gloutT = nc.dram_tensor("gloutT", [D, D], f16, kind="ExternalInput")
    d_ffw1T = nc.dram_tensor("ffw1T", [D, FFN], f16, kind="ExternalInput")
    d_ffw2T = nc.dram_tensor("ffw2T", [FFN // 2, 2 * D], f16, kind="ExternalInput")
    d_wgs = nc.dram_tensor("wgs", [D, C], f16, kind="ExternalInput")
    d_lsw2 = nc.dram_tensor("lsw2", [D, C], f16, kind="ExternalInput")
    d_id = nc.dram_tensor("idm", [128, 128], f32, kind="ExternalInput")
    d_id16 = nc.dram_tensor("idm16", [128, 128], f16, kind="ExternalInput")
    d_chc = nc.dram_tensor("chc", [D, 4], f32, kind="ExternalInput")      # [a,1-a,v0,z0b]
    d_ccc = nc.dram_tensor("ccc", [C, 3], f32, kind="ExternalInput")      # [a,1-a,v0]
    d_g0 = nc.dram_tensor("g0", [1, D], f32, kind="ExternalInput")        # growth row 0
    if has_gob:
        d_ones = nc.dram_tensor("onesr", [1, D], f16, kind="ExternalInput")
        d_gob = nc.dram_tensor("gob", [1, D], f16, kind="ExternalInput")
    if has_bu:
        d_ones2 = nc.dram_tensor("onesr2", [1, D], f16, kind="ExternalInput")
        d_bu = nc.dram_tensor("bu", [1, C], f16, kind="ExternalInput")
    if has_ffb:
        d_fb = nc.dram_tensor("fb", [FFN, 1], f32, kind="ExternalInput")

    d_out_res = nc.dram_tensor("out_res", [T, D], f32, kind="ExternalOutput")
    d_out_level = nc.dram_tensor("out_level", [T, C], f32, kind="ExternalOutput")
    d_out_growth = nc.dram_tensor("out_growth", [T + 1, D], f32, kind="ExternalOutput")
    d_out_season = nc.dram_tensor("out_season", [T + PRED, D], f32, kind="ExternalOutput")

    from contextlib import ExitStack
    with tile.TileContext(nc) as tc, ExitStack() as _es:
        cp = _es.enter_context(tc.tile_pool(name="cp", bufs=1))
        sp = _es.enter_context(tc.tile_pool(name="sp", bufs=6))
        so = _es.enter_context(tc.tile_pool(name="so", bufs=2))
        pp = _es.enter_context(tc.tile_pool(name="pp", bufs=6, space="PSUM"))
        pq = _es.enter_context(tc.tile_pool(name="pq", bufs=2, space="PSUM"))

        dma = nc.sync.dma_start

        eps_col = cp.tile([128, 1], f32, tag="eps")
        nc.vector.memset(eps_col[:], EPS)

        # ======== stage-0 DMAs (front-of-queue: what the PE needs first) ====
        # even/odd rows of res into the slots later reused by sigmoid tiles
        d_res_eo = d_res.rearrange("(a two) d -> a two d", two=2)
        xe_sb, xo_sb = [], []
        for j in range(4):
            t_ = cp.tile([128, D], f32, tag=f"sg{j}")
            dma(out=t_[:], in_=d_res_eo[j * 128:(j + 1) * 128, 0, :])
            xe_sb.append(t_)
        for j in range(4):
            t_ = cp.tile([128, D], f32, tag=f"sg{4 + j}")
            dma(out=t_[:], in_=d_res_eo[j * 128:(j + 1) * 128, 1, :])
            xo_sb.append(t_)

        c512_sb, s512_sb = [], []
        for kk in range(4):
            t_ = cp.tile([128, 514], f32, tag=f"e5{kk}")
            dma(out=t_[:], in_=d_cs512[kk * 128:(kk + 1) * 128, :])
            c512_sb.append(t_[:, 0:257])
            s512_sb.append(t_[:, 257:514])
        oc_sb, os_sb = [], []
        for kk in range(4):
            t_ = cp.tile([128, 514], f32, tag=f"o5{kk}")
            dma(out=t_[:], in_=d_os512[kk * 128:(kk + 1) * 128, :])
            oc_sb.append(t_[:, 0:257])
            os_sb.append(t_[:, 257:514])

        def bcast_row(dram, tag, n):
            t_ = cp.tile([128, n], f32, tag=tag)
            dma(out=t_[:], in_=dram[0:1, :].broadcast_to((128, n)))
            return t_

        id_sb = cp.tile([128, 128], f32, tag="id")
        dma(out=id_sb[:], in_=d_id[:])
        id16_sb = cp.tile([128, 128], f16, tag="id16")
        dma(out=id16_sb[:], in_=d_id16[:])

        # remaining inputs (ordered roughly by first use); issue on the
        # gpsimd queue so the sync queue stays clear for the critical path,
        # and gate them behind the first E/O matmul chain so the rfft inputs
        # get full DMA bandwidth at kernel start
        _gated = []

        def gdma(out, in_):
            bi = nc.gpsimd.dma_start(out=out, in_=in_)
            _gated.append(bi)
            return bi
        res_sb = []
        for j in range(NT):
            t_ = cp.tile([128, D], f32, tag=f"res{j}")
            gdma(out=t_[:], in_=d_res[j * 128:(j + 1) * 128, :])
            res_sb.append(t_)

        cs_sb, snn_sb = [], []
        for i in range(4):
            kw = KB[i + 1] - KB[i]
            t_ = cp.tile([128, T], f16, tag=f"cs{i}")
            gdma(out=t_[0:kw, :], in_=d_cs[KB[i]:KB[i + 1], :])
            cs_sb.append(t_)
        for i in range(4):
            kw = KB[i + 1] - KB[i]
            t_ = cp.tile([128, T], f16, tag=f"sn{i}")
            gdma(out=t_[0:kw, :], in_=d_snn[KB[i]:KB[i + 1], :])
            snn_sb.append(t_)

        glinT_sb, gloutT_sb = [], []
        for i in range(ND):
            t_ = cp.tile([128, D], f16, tag=f"gin{i}")
            gdma(out=t_[:], in_=d_glinT[i * 128:(i + 1) * 128, :])
            glinT_sb.append(t_)
        for i in range(ND):
            t_ = cp.tile([128, D], f16, tag=f"got{i}")
            gdma(out=t_[:], in_=d_gloutT[i * 128:(i + 1) * 128, :])
            gloutT_sb.append(t_)

        wgs_sb, lsw2_sb = [], []
        for i in range(ND):
            t_ = cp.tile([128, C], f16, tag=f"wgs{i}")
            gdma(out=t_[:], in_=d_wgs[i * 128:(i + 1) * 128, :])
            wgs_sb.append(t_)
        for i in range(ND):
            t_ = cp.tile([128, C], f16, tag=f"lsw{i}")
            gdma(out=t_[:], in_=d_lsw2[i * 128:(i + 1) * 128, :])
            lsw2_sb.append(t_)

        chc_sb = []   # per ch-tile: cols [alpha, 1-alpha, v0, z0b]
        for m in range(ND):
            t_ = cp.tile([128, 4], f32, tag=f"chc{m}")
            gdma(out=t_[:], in_=d_chc[m * 128:(m + 1) * 128, :])
            chc_sb.append(t_)
        ccc_sb = cp.tile([C, 3], f32, tag="ccc")
        gdma(out=ccc_sb[:], in_=d_ccc[:])

        lvl_sb = []
        for j in range(NT):
            t_ = cp.tile([128, C], f32, tag=f"lvl{j}")
            gdma(out=t_[:], in_=d_level[j * 128:(j + 1) * 128, :])
            lvl_sb.append(t_)

        if has_gob:
            ones_sb = cp.tile([1, D], f16, tag="ones")
            dma(out=ones_sb[:], in_=d_ones[:])
            gob_sb = cp.tile([1, D], f16, tag="gob")
            dma(out=gob_sb[:], in_=d_gob[:])
        if has_bu:
            ones2_sb = cp.tile([1, D], f16, tag="ones2")
            dma(out=ones2_sb[:], in_=d_ones2[:])
            bu_sb = cp.tile([1, C], f16, tag="bu")
            dma(out=bu_sb[:], in_=d_bu[:])
        if has_ffb:
            fb_sb = cp.tile([128, NF], f32, tag="fb")
            for fi in range(NF):
                dma(out=fb_sb[:, fi:fi + 1], in_=d_fb[fi * 128:(fi + 1) * 128, :])

        # growth row 0 is input-independent (v0 @ W + b): DMA straight through
        dma(out=d_out_growth[0:1, :], in_=d_g0[:])

        # level input transpose (PE filler while the rfft waits on DMA)
        lvT = cp.tile([64, T], f16, tag="lvT")
        for j in range(NT):
            ps = pq.tile([128, 512], f32, tag="s")
            nc.tensor.transpose(ps[0:C, 0:128], lvl_sb[j][:, 0:C], id_sb[:])
            nc.scalar.copy(lvT[:, j * 128:(j + 1) * 128], ps[0:C, 0:128])

        # ======= S1: split-DFT (E/O bins 0..256, fp32) + twiddle combine ====
        # ======= S2: top-8 mask per d -> MR/MI [d, k] (f16) =================
        mr_sb, mi_sb = [], []
        for i in range(ND):
            psER = pp.tile([128, 512], f32, tag="m")
            psEI = pp.tile([128, 512], f32, tag="m")
            psOR = pp.tile([128, 512], f32, tag="m")
            psOI = pp.tile([128, 512], f32, tag="m")
            for (ps, src, tab) in ((psER, xe_sb, c512_sb), (psEI, xe_sb, s512_sb),
                                   (psOR, xo_sb, oc_sb), (psOI, xo_sb, os_sb)):
                for kk in range(4):
                    bi = nc.tensor.matmul(
                        ps[:, 0:257], src[kk][:, i * 128:(i + 1) * 128],
                        tab[kk], start=(kk == 0), stop=(kk == 3))
            if i == 0 and _gated:
                from concourse.tile_rust import add_dep_helper
                for g in _gated:
                    add_dep_helper(g.ins, bi.ins,
                                   reason="bulk DMA yields to rfft inputs")
                _gated.clear()

            xr = cp.tile([128, 512], f32, tag=("xr0" if i % 2 == 0 else "e50"))
            xi = cp.tile([128, 512], f32, tag=("xi0" if i % 2 == 0 else "e51"))
            amp = cp.tile([128, 512], f32, tag=f"amp{i % 2}")
            rep = cp.tile([128, 512], f32, tag=f"rep{i % 2}")
            TT = nc.vector.tensor_tensor
            # odd tables carry the twiddle; E mirrors by conjugate symmetry.
            # lo bins 1..256 -> cols 0..255; hi bins (reversed) store the
            # NEGATED imag part; snn rows 256+ are negated on host to match.
            # (only one PSUM operand allowed per DVE op: evac the odd pair)
            nc.scalar.copy(amp[:, 0:257], psOR[:, 0:257])
            nc.scalar.copy(rep[:, 0:257], psOI[:, 0:257])
            TT(xr[:, 0:256], psER[:, 1:257], amp[:, 1:257], AL.add)
            TT(xr[:, 256:511], psER[:, 255:0:-1], amp[:, 255:0:-1], AL.subtract)
            TT(xi[:, 0:256], psEI[:, 1:257], rep[:, 1:257], AL.add)
            TT(xi[:, 256:511], psEI[:, 255:0:-1], rep[:, 255:0:-1], AL.subtract)

            # amplitude^2 and top-8 mask
            nc.scalar.activation(amp[:, 0:F], xr[:, 0:F], AF.Square)
            nc.scalar.activation(rep[:, 0:F], xi[:, 0:F], AF.Square)
            TT(amp[:, 0:F], amp[:, 0:F], rep[:, 0:F], AL.add)
            mx8 = sp.tile([128, 8], f32, tag="mx8")
            nc.vector.max(mx8[:], amp[:, 0:F])
            # top-8 selection as a threshold on the 8th-largest amplitude,
            # fused into the masking multiplies
            mr = cp.tile([128, 512], f16, tag=f"mr{i}")
            mi = cp.tile([128, 512], f16, tag=f"mi{i}")
            nc.vector.scalar_tensor_tensor(mr[:, 0:F], amp[:, 0:F],
                                           mx8[:, 7:8], xr[:, 0:F],
                                           AL.is_ge, AL.mult)
            nc.vector.scalar_tensor_tensor(mi[:, 0:F], amp[:, 0:F],
                                           mx8[:, 7:8], xi[:, 0:F],
                                           AL.is_ge, AL.mult)
            mr_sb.append(mr)
            mi_sb.append(mi)

        # ======= S3: MRt/MIt [k,d] (f16) and MRW/MIW [k,c] (f16) =========
        mrt_sb, mit_sb = [], []
        for kk in range(4):
            kw = KB[kk + 1] - KB[kk]
            for (src, dstl, tg) in ((mr_sb, mrt_sb, "mrt"), (mi_sb, mit_sb, "mit")):
                ps = pp.tile([128, 512], f32, tag="m")
                for i in range(ND):
                    nc.tensor.matmul(
                        ps[0:kw, i * 128:(i + 1) * 128],
                        src[i][:, KB[kk]:KB[kk + 1]], id16_sb[:],
                        start=True, stop=True)
                t_ = cp.tile([128, 512], f16, tag=f"{tg}{kk}")
                nc.scalar.copy(t_[0:kw, :], ps[0:kw, :])
                dstl.append(t_)

        mrw_sb, miw_sb = [], []
        for kk in range(4):
            kw = KB[kk + 1] - KB[kk]
            for (src, dstl, tg) in ((mr_sb, mrw_sb, "mrw"), (mi_sb, miw_sb, "miw")):
                ps = pq.tile([128, 512], f32, tag="s")
                for i in range(ND):
                    nc.tensor.matmul(
                        ps[0:kw, 0:C], src[i][:, KB[kk]:KB[kk + 1]],
                        lsw2_sb[i][:], start=(i == 0), stop=(i == ND - 1))
                t_ = cp.tile([128, C], f16, tag=f"{tg}{kk}")
                nc.scalar.copy(t_[0:kw, :], ps[0:kw, 0:C])
                dstl.append(t_)

        # ======= S4: season [tau,d]; res2 = res - season; season out =====
        sea_sb, res2_sb = [], []
        for j in range(NT):
            ps = pp.tile([128, 512], f32, tag="m")
            for kk in range(4):
                kw = KB[kk + 1] - KB[kk]
                nc.tensor.matmul(
                    ps[:], cs_sb[kk][0:kw, j * 128:(j + 1) * 128],
                    mrt_sb[kk][0:kw, 0:D], start=(kk == 0), stop=False)
                nc.tensor.matmul(
                    ps[:], snn_sb[kk][0:kw, j * 128:(j + 1) * 128],
                    mit_sb[kk][0:kw, 0:D], start=False, stop=(kk == 3))
            sea = cp.tile([128, D], f32, tag=f"sea{j % 4}")
            nc.scalar.copy(sea[:], ps[:])
            r2 = cp.tile([128, D], f32, tag=f"r2{j}")
            nc.vector.tensor_tensor(r2[:], res_sb[j][:], ps[:], AL.subtract)
            sea_sb.append(sea)
            res2_sb.append(r2)
            dma(out=d_out_season[j * 128:(j + 1) * 128, :], in_=sea[:])
            if j < 2:   # periodic extension: rows 1024..1279 = rows 0..255
                dma(out=d_out_season[T + j * 128:T + (j + 1) * 128, :], in_=sea[:])

        # ======= S5: res2T [d,t] (f16) ===================================
        res2T_sb = []
        for i in range(ND):
            t_ = cp.tile([128, T], f16, tag=f"r2t{i}")
            for jh in range(2):
                ps = pp.tile([128, 512], f32, tag="m")
                for j4 in range(4):
                    j = jh * 4 + j4
                    nc.tensor.transpose(
                        ps[:, j4 * 128:(j4 + 1) * 128],
                        res2_sb[j][:, i * 128:(i + 1) * 128], id_sb[:])
                nc.scalar.copy(t_[:, jh * 512:(jh + 1) * 512], ps[:])
            res2T_sb.append(t_)

        # ======= S6: vT [ch,t] = glinT.T @ res2T  (into vd slots) ========
        vT_sb = []
        for m in range(ND):
            t_ = cp.tile([128, T], f32, tag=f"vd{m}")
            for th in range(2):
                ps = pp.tile([128, 512], f32, tag="m")
                for i in range(ND):
                    nc.tensor.matmul(
                        ps[:], glinT_sb[i][:, m * 128:(m + 1) * 128],
                        res2T_sb[i][:, th * 512:(th + 1) * 512],
                        start=(i == 0), stop=(i == ND - 1))
                nc.scalar.copy(t_[:, th * 512:(th + 1) * 512], ps[:])
            vT_sb.append(t_)

        # ======= S7: vdiff, u=(1-a)*vd, EMA scan -> sT [ch, 1+t] (f16) ===
        sT_sb = []
        for m in range(ND):
            vd = cp.tile([128, T], f32, tag=f"r2t{m}")
            nc.vector.tensor_tensor(vd[:, 1:T], vT_sb[m][:, 1:T],
                                    vT_sb[m][:, 0:T - 1], AL.subtract)
            nc.vector.tensor_tensor(vd[:, 0:1], vT_sb[m][:, 0:1],
                                    chc_sb[m][:, 3:4], AL.subtract)
            u = vT_sb[m]   # overwrite vT slot elementwise from vd
            nc.vector.tensor_scalar(u[:], vd[:], chc_sb[m][:, 1:2], None, AL.mult)
            st = cp.tile([128, 1056], f16, tag=f"st{m}")
            nc.vector.tensor_copy(st[:, 0:1], chc_sb[m][:, 2:3])
            nc.vector.tensor_tensor_scan(
                st[:, 1:T + 1], chc_sb[m][:, 0:1].broadcast_to((128, T)), u[:],
                chc_sb[m][:, 2:3], AL.mult, AL.add)
            sT_sb.append(st)

        # ======= level path: u-accum [c,t], scan, out ====================
        usb = cp.tile([64, T], f32, tag="r2t0")
        lvs = cp.tile([64, T], f32, tag="r2t1")
        for th in range(2):
            ps = pq.tile([128, 512], f32, tag="s")
            for kk in range(4):
                kw = KB[kk + 1] - KB[kk]
                nc.tensor.matmul(ps[0:C, :], mrw_sb[kk][0:kw, :],
                                 cs_sb[kk][0:kw, th * 512:(th + 1) * 512],
                                 start=(kk == 0), stop=False)
                nc.tensor.matmul(ps[0:C, :], miw_sb[kk][0:kw, :],
                                 snn_sb[kk][0:kw, th * 512:(th + 1) * 512],
                                 start=False, stop=False)
            for m in range(ND):
                nc.tensor.matmul(ps[0:C, :], wgs_sb[m][:],
                                 sT_sb[m][:, th * 512:(th + 1) * 512],
                                 start=False, stop=(m == ND - 1 and not has_bu))
            if has_bu:
                nc.tensor.matmul(ps[0:C, :], bu_sb[:], ones2_sb[:],
                                 start=False, stop=True)
            nc.vector.scalar_tensor_tensor(
                usb[:, th * 512:(th + 1) * 512], lvT[:, th * 512:(th + 1) * 512],
                ccc_sb[:, 1:2], ps[0:C, :], AL.mult, AL.add)
        nc.vector.tensor_tensor_scan(
            lvs[:], ccc_sb[:, 0:1].broadcast_to((64, T)), usb[:],
            ccc_sb[:, 2:3], AL.mult, AL.add)
        for j in range(NT):
            ps = pq.tile([128, 512], f32, tag="s")
            nc.tensor.transpose(ps[:, 0:C], lvs[:, j * 128:(j + 1) * 128],
                                id_sb[0:64, 0:64])
            lo = so.tile([128, C], f32, tag="lvo")
            nc.scalar.copy(lo[:], ps[:, 0:C])
            dma(out=d_out_level[j * 128:(j + 1) * 128, :], in_=lo[:])

        # FF weights arrive into slots freed by the level/synthesis stages
        ffw1_sb = []
        for i in range(ND):
            for h in range(2):
                t_ = cp.tile([128, T], f16, tag=(f"cs{i}" if h == 0 else f"sn{i}"))
                gdma(out=t_[:], in_=d_ffw1T[i * 128:(i + 1) * 128,
                                           h * 1024:(h + 1) * 1024])
                ffw1_sb.append(t_)  # index 2*i + h

        ffw2_sb = []
        for f in range(8):
            t_ = cp.tile([128, 1024], f16, tag=f"ff2{f}")
            gdma(out=t_[:], in_=d_ffw2T[f * 128:(f + 1) * 128, :])
            ffw2_sb.append(t_)

        # ======= S8: growth rows 1..1024; x1 = res2 - growth[1:] =========
        x1_sb = []
        for j in range(NT):
            ps = pp.tile([128, 512], f32, tag="m")
            for m in range(ND):
                nc.tensor.matmul(
                    ps[:], sT_sb[m][:, j * 128 + 1:(j + 1) * 128 + 1],
                    gloutT_sb[m][:], start=(m == 0),
                    stop=(m == ND - 1 and not has_gob))
            if has_gob:
                nc.tensor.matmul(ps[:], ones_sb[0:1, 0:128], gob_sb[:],
                                 start=False, stop=True)
            x1 = cp.tile([128, D], f32, tag=f"sea{j % 4}")
            nc.vector.tensor_tensor(x1[:], res2_sb[j][:], ps[:], AL.subtract)
            gr = cp.tile([128, D], f32, tag=f"r2{j}")
            nc.scalar.copy(gr[:], ps[:])
            dma(out=d_out_growth[j * 128 + 1:(j + 1) * 128 + 1, :], in_=gr[:])
            x1_sb.append(x1)

        # ======= layer norm: z = (x - mean) * rstd (gamma/beta folded) ===
        def norm_z(x_in, out_tile, j):
            st6 = sp.tile([128, 6], f32, tag="st6")
            nc.vector.bn_stats(st6[:], x_in[:])
            mv = sp.tile([128, 2], f32, tag="mv")
            nc.vector.bn_aggr(mv[:], st6[:])
            std = sp.tile([128, 1], f32, tag="col")
            nc.scalar.activation(std[:], mv[:, 1:2], AF.Sqrt, bias=eps_col[:])
            rstd = sp.tile([128, 1], f32, tag="col")
            nc.vector.reciprocal(rstd[:], std[:])
            nc.vector.tensor_scalar(out_tile[:], x_in[:], mv[:, 0:1], rstd[:],
                                    AL.subtract, AL.mult)
            return out_tile

        # ======= S9: norm1 -> z1 [t,d] (gamma1 folded into ffw1) =========
        res3_sb = []
        for j in range(NT):
            out = cp.tile([128, D], f32, tag=f"res{j}")
            norm_z(x1_sb[j], out, j)
            res3_sb.append(out)

        # ======= S10: res3T [d,t] (f16) ==================================
        res3T_sb = []
        for i in range(ND):
            t_ = cp.tile([128, T], f16, tag=f"vd{i}")
            for jh in range(2):
                ps = pp.tile([128, 512], f32, tag="m")
                for j4 in range(4):
                    j = jh * 4 + j4
                    nc.tensor.transpose(
                        ps[:, j4 * 128:(j4 + 1) * 128],
                        res3_sb[j][:, i * 128:(i + 1) * 128], id_sb[:])
                nc.scalar.copy(t_[:, jh * 512:(jh + 1) * 512], ps[:])
            res3T_sb.append(t_)

        # ======= S11: FF1 (gamma1-scaled weights) + sigmoid(+bias) =======
        sig_sb = []
        for fi in range(NF):
            h, fo = fi // 8, fi % 8
            sg = cp.tile([128, T], f16, tag=f"sg{fi}")
            for th in range(2):
                ps = pp.tile([128, 512], f32, tag="m")
                for i in range(ND):
                    nc.tensor.matmul(
                        ps[:], ffw1_sb[2 * i + h][:, fo * 128:(fo + 1) * 128],
                        res3T_sb[i][:, th * 512:(th + 1) * 512],
                        start=(i == 0), stop=(i == ND - 1))
                if has_ffb:
                    nc.scalar.activation(sg[:, th * 512:(th + 1) * 512], ps[:],
                                         AF.Sigmoid, bias=fb_sb[:, fi:fi + 1])
                else:
                    nc.scalar.activation(sg[:, th * 512:(th + 1) * 512], ps[:],
                                         AF.Sigmoid)
            sig_sb.append(sg)

        # ======= S12/S13: FF2 + residual + norm2 -> out ==================
        for j in range(NT):
            ps = pp.tile([128, 512], f32, tag="m")
            for fi in range(NF):
                nc.tensor.matmul(
                    ps[:], sig_sb[fi][:, j * 128:(j + 1) * 128],
                    ffw2_sb[fi % 8][:, (fi // 8) * 512:(fi // 8 + 1) * 512],
                    start=(fi == 0), stop=(fi == NF - 1))
            u2 = cp.tile([128, D], f32, tag=f"sea{j % 4}")
            nc.vector.tensor_tensor(u2[:], res3_sb[j][:], ps[:], AL.add)
            out = cp.tile([128, D], f32, tag=f"st{j % 4}")
            norm_z(u2, out, j)
            dma(out=d_out_res[j * 128:(j + 1) * 128, :], in_=out[:])

    nc.compile()
    return nc


def _host_prep(inputs):
    """Build per-core input maps (numpy only)."""
    def sig(x):
        return 1.0 / (1.0 + np.exp(-x.astype(np.float64)))

    res = np.ascontiguousarray(inputs["res"], dtype=np.float32)
    level = np.ascontiguousarray(inputs["level"], dtype=np.float32)

    tp = np.arange(512)
    k2 = np.arange(257)
    ang_e = 2.0 * np.pi * np.outer(2 * tp, k2) / T
    ang_o = 2.0 * np.pi * np.outer(2 * tp + 1, k2) / T
    cs512 = np.concatenate(
        [np.cos(ang_e), -np.sin(ang_e)], axis=1).astype(np.float32)  # (512, 514)
    os512 = np.concatenate(
        [np.cos(ang_o), -np.sin(ang_o)], axis=1).astype(np.float32)

    t = np.arange(T)
    k = np.arange(1, F + 1)
    ang_kt = 2.0 * np.pi * np.outer(k, t) / T
    cs = (np.cos(ang_kt) * (2.0 / T)).astype(np.float16)
    snn_f = -np.sin(ang_kt) * (2.0 / T)
    snn_f[256:] = -snn_f[256:]   # hi bins store negated imag part on device
    snn = snn_f.astype(np.float16)

    gl_in_w = inputs["gl_in_w"].astype(np.float64)
    gl_out_w = inputs["gl_out_w"].astype(np.float64)
    alpha_ch = np.repeat(sig(inputs["gl_sw"]).reshape(-1), 64)      # (512,)
    v0_ch = inputs["gl_v0"].reshape(-1).astype(np.float64)
    z0b = (inputs["gl_z0"].reshape(-1).astype(np.float64)
           - inputs["gl_in_b"].astype(np.float64))

    alpha_c = sig(inputs["ll_sw"]).reshape(-1)                      # (64,)
    ll_gw = inputs["ll_gw"].astype(np.float64)
    ll_sw2 = inputs["ll_sw2"].astype(np.float64)
    b_g = (inputs["gl_out_b"].astype(np.float64) @ ll_gw.T
           + inputs["ll_gb"].astype(np.float64))
    wgs = (gl_out_w.T @ ll_gw.T) * alpha_c[None, :]
    lsw2 = ll_sw2.T * (-(1.0 - alpha_c))[None, :]
    bias_u = (-(1.0 - alpha_c) * inputs["ll_sb"].astype(np.float64)
              + alpha_c * b_g)

    chc = np.stack([alpha_ch, 1.0 - alpha_ch, v0_ch, z0b], axis=1)
    ccc = np.stack([alpha_c, 1.0 - alpha_c,
                    inputs["ll_v0"].reshape(-1).astype(np.float64)], axis=1)
    g0 = v0_ch @ gl_out_w.T + inputs["gl_out_b"].astype(np.float64)

    # layernorm gamma/beta folding: norm1's gamma is absorbed into ff_w1 and
    # its beta into the sigmoid bias. The residual/norm2 gamma/beta must be
    # identity for this build (true for the reference model: gamma=1, beta=0).
    n1g = inputs["n1_g"].astype(np.float64)
    n1b = inputs["n1_b"].astype(np.float64)
    assert np.all(n1g == 1.0) and np.all(inputs["n2_g"] == 1.0), \
        "non-identity layernorm gamma not supported by this build"
    assert np.all(n1b == 0.0) and np.all(inputs["n2_b"] == 0.0), \
        "nonzero layernorm beta not supported by this build"
    ffw1 = inputs["ff_w1"].astype(np.float64) * n1g[None, :]
    ffb = ffw1 @ n1b   # (FFN,) sigmoid bias

    has_gob = bool(np.any(inputs["gl_out_b"] != 0))
    has_bu = bool(np.any(bias_u != 0))
    has_ffb = bool(np.any(ffb != 0))

    shared = {
        "cs512": cs512, "os512": os512, "cs": cs, "snn": snn,
        "glinT": np.ascontiguousarray(gl_in_w.T, dtype=np.float16),
        "gloutT": np.ascontiguousarray(gl_out_w.T, dtype=np.float16),
        "ffw1T": np.ascontiguousarray(ffw1.T, dtype=np.float16),
                "ffw2T": np.ascontiguousarray(
            np.concatenate([inputs["ff_w2"].T[:FFN // 2],
                            inputs["ff_w2"].T[FFN // 2:]], axis=1),
            dtype=np.float16),
        "wgs": wgs.astype(np.float16),
        "lsw2": lsw2.astype(np.float16),
        "idm": np.eye(128, dtype=np.float32),
        "idm16": np.eye(128, dtype=np.float16),
        "chc": chc.astype(np.float32),
        "ccc": ccc.astype(np.float32),
        "g0": g0.astype(np.float32).reshape(1, D),
    }
    if has_gob:
        shared["onesr"] = np.ones((1, D), dtype=np.float16)
        shared["gob"] = inputs["gl_out_b"].astype(np.float16).reshape(1, D)
    if has_bu:
        shared["onesr2"] = np.ones((1, D), dtype=np.float16)
        shared["bu"] = bias_u.astype(np.float16).reshape(1, C)
    if has_ffb:
        shared["fb"] = ffb.astype(np.float32).reshape(FFN, 1)

    in_maps = []
    for b in range(res.shape[0]):
        m = dict(shared)
        m["res"] = res[b]
        m["level"] = level[b]
        in_maps.append(m)
    return in_maps, (has_gob, has_bu, has_ffb)


def kernel(**inputs):
    _ensure_axon_hooks()
    from concourse.bass_utils import run_bass_kernel_spmd

    in_maps, flags = _host_prep(inputs)
    key = ("nc", flags)
    if key not in _CACHE:
        _CACHE[key] = _build_program(flags)
    nc = _CACHE[key]

    n = len(in_maps)
    kw = {}
    if os.environ.get("KERNEL_TRACE"):
        kw = dict(trace=True, tmpdir=os.environ.get("KERNEL_TRACE_DIR") or None)
    r_ = None
    for attempt in range(3):
        try:
            r_ = run_bass_kernel_spmd(nc, in_maps, list(range(n)), **kw)
            break
        except Exception:
            if attempt == 2:
                raise
            import time
            time.sleep(2.0)
    _CACHE["last_exec_time_ns"] = r_.exec_time_ns

    res_out = np.stack([r_.results[b]["out_res"] for b in range(n)])
    level_out = np.stack([r_.results[b]["out_level"] for b in range(n)])
    growth_out = np.stack([r_.results[b]["out_growth"] for b in range(n)])
    season_out = np.stack([r_.results[b]["out_season"] for b in range(n)])
    return (res_out.astype(np.float32), level_out.astype(np.float32),
            growth_out.astype(np.float32), season_out.astype(np.float32))


# revision 28
# speedup vs baseline: 1.0684x; 1.0044x over previous
"""Trainium2 Bass kernel for nn_EncoderLayer_45423574122725.

Data-parallel over batch: 8 batch elements -> 8 NeuronCores, full pipeline
per core:
  radix-2 split-DFT rfft (fp32 matmuls + DVE twiddle combine) -> top-8 bins
  per (b,d) via DVE max8/match_replace -> masked-spectrum trig resynthesis
  (f16 matmuls) -> growth layer (matmul + first-diff + EMA via
  tensor_tensor_scan) -> layernorm -> sigmoid FF -> layernorm -> level layer
  (fused matmuls + EMA scan).

The FFT smoothing convs in the reference are exact exponential moving
averages (verified algebraically + numerically), implemented with the DVE
tensor_tensor_scan recurrence  state = a*state + b  along the free dim.
"""
import os
import sys
import types

sys.path.insert(0, "/opt/trn_rl_repo")

import numpy as np

import concourse.bacc as bacc
import concourse.bass as bass
import concourse.mybir as mybir
from concourse import tile

f32 = mybir.dt.float32
f16 = mybir.dt.float16
AL = mybir.AluOpType
AF = mybir.ActivationFunctionType
AX = mybir.AxisListType

T = 1024          # seq len
D = 512           # d_model
F = 511           # rfft bins 1..511 (LOW_FREQ=1, Nyquist excluded)
PRED = 256
C = 64            # level channels
FFN = 2048
EPS = 1e-5
NT = T // 128     # 8 time tiles
ND = D // 128     # 4 feature tiles
NF = FFN // 128   # 16 ffn tiles
KB = [0, 128, 256, 384, 511]   # bin-tile boundaries (bin = col+1)

_CACHE: dict = {}


def _ensure_axon_hooks():
    """Install the NTFF profile hook registry if the image's antenv lacks it."""
    try:
        from antenv.axon_hooks import get_axon_ntff_profile_hook  # noqa: F401
        return
    except ImportError:
        pass
    import antenv

    mod = types.ModuleType("antenv.axon_hooks")
    _h = [None]

    def _set(h):
        _h[0] = h

    def _get():
        return _h[0]

    mod.set_axon_ntff_profile_hook = _set
    mod.get_axon_ntff_profile_hook = _get
    sys.modules["antenv.axon_hooks"] = mod
    antenv.axon_hooks = mod
    try:
        from trn_agent_boot.trn_boot import _ntff_profile_via_ctypes
        _set(_ntff_profile_via_ctypes("/opt/axon/libaxon_pjrt.so"))
    except Exception:
        pass


def _build_program(flags):
    """Emit the single-core Bass/Tile program (SPMD across 8 cores).

    flags: (has_gob, has_bu, has_ffb) — whether those bias terms are nonzero.
    """
    has_gob, has_bu, has_ffb = flags
    from concourse import tile_utils
    tile_utils.max_sbuf_usage = 208 * 1024  # cayman usable; default cap is stale

    nc = bacc.Bacc("TRN2", target_bir_lowering=False, debug=False)

    # ---------------- DRAM I/O ----------------
    d_res = nc.dram_tensor("res", [T, D], f32, kind="ExternalInput")
    d_level = nc.dram_tensor("level", [T, C], f32, kind="ExternalInput")
    d_cs512 = nc.dram_tensor("cs512", [512, 514], f32, kind="ExternalInput")
    d_os512 = nc.dram_tensor("os512", [512, 514], f32, kind="ExternalInput")
    d_cs = nc.dram_tensor("cs", [F, T], f16, kind="ExternalInput")        # cos * 2/T
    d_snn = nc.dram_tensor("snn", [F, T], f16, kind="ExternalInput")      # +-sin * 2/T
    d_glinT = nc.dram_tensor("glinT", [D, D], f16, kind="ExternalInput")  # gl_in_w.T
    d_gloutT = nc.dram_tensor("gloutT", [D, D], f16, kind="ExternalInput")
    d_ffw1T = nc.dram_tensor("ffw1T", [D, FFN], f16, kind="ExternalInput")
    d_ffw2T = nc.dram_tensor("ffw2T", [FFN // 2, 2 * D], f16, kind="ExternalInput")
    d_wgs = nc.dram_tensor("wgs", [D, C], f16, kind="ExternalInput")
    d_lsw2 = nc.dram_tensor("lsw2", [D, C], f16, kind="ExternalInput")
    d_id = nc.dram_tensor("idm", [128, 128], f32, kind="ExternalInput")
    d_id16 = nc.dram_tensor("idm16", [128, 128], f16, kind="ExternalInput")
    d_chc = nc.dram_tensor("chc", [D, 4], f32, kind="ExternalInput")      # [a,1-a,v0,z0b]
    d_ccc = nc.dram_tensor("ccc", [C, 3], f32, kind="ExternalInput")      # [a,1-a,v0]
    d_g0 = nc.dram_tensor("g0", [1, D], f32, kind="ExternalInput")        # growth row 0
    if has_gob:
        d_ones = nc.dram_tensor("onesr", [1, D], f16, kind="ExternalInput")
        d_gob = nc.dram_tensor("gob", [1, D], f16, kind="ExternalInput")
    if has_bu:
        d_ones2 = nc.dram_tensor("onesr2", [1, D], f16, kind="ExternalInput")
        d_bu = nc.dram_tensor("bu", [1, C], f16, kind="ExternalInput")
    if has_ffb:
        d_fb = nc.dram_tensor("fb", [FFN, 1], f32, kind="ExternalInput")

    d_out_res = nc.dram_tensor("out_res", [T, D], f32, kind="ExternalOutput")
    d_out_level = nc.dram_tensor("out_level", [T, C], f32, kind="ExternalOutput")
    d_out_growth = nc.dram_tensor("out_growth", [T + 1, D], f32, kind="ExternalOutput")
    d_out_season = nc.dram_tensor("out_season", [T + PRED, D], f32, kind="ExternalOutput")

    from contextlib import ExitStack
    with tile.TileContext(nc) as tc, ExitStack() as _es:
        cp = _es.enter_context(tc.tile_pool(name="cp", bufs=1))
        sp = _es.enter_context(tc.tile_pool(name="sp", bufs=6))
        so = _es.enter_context(tc.tile_pool(name="so", bufs=2))
        pp = _es.enter_context(tc.tile_pool(name="pp", bufs=6, space="PSUM"))
        pq = _es.enter_context(tc.tile_pool(name="pq", bufs=2, space="PSUM"))

        dma = nc.sync.dma_start

        eps_col = cp.tile([128, 1], f32, tag="eps")
        nc.vector.memset(eps_col[:], EPS)

        # ======== stage-0 DMAs (front-of-queue: what the PE needs first) ====
        # even/odd rows of res into the slots later reused by sigmoid tiles
        d_res_eo = d_res.rearrange("(a two) d -> a two d", two=2)
        xe_sb, xo_sb = [], []
        for j in range(4):
            t_ = cp.tile([128, D], f32, tag=f"sg{j}")
            dma(out=t_[:], in_=d_res_eo[j * 128:(j + 1) * 128, 0, :])
            xe_sb.append(t_)
        for j in range(4):
            t_ = cp.tile([128, D], f32, tag=f"sg{4 + j}")
            dma(out=t_[:], in_=d_res_eo[j * 128:(j + 1) * 128, 1, :])
            xo_sb.append(t_)

        c512_sb, s512_sb = [], []
        for kk in range(4):
            t_ = cp.tile([128, 514], f32, tag=f"e5{kk}")
            dma(out=t_[:], in_=d_cs512[kk * 128:(kk + 1) * 128, :])
            c512_sb.append(t_[:, 0:257])
            s512_sb.append(t_[:, 257:514])
        oc_sb, os_sb = [], []
        for kk in range(4):
            t_ = cp.tile([128, 514], f32, tag=f"o5{kk}")
            dma(out=t_[:], in_=d_os512[kk * 128:(kk + 1) * 128, :])
            oc_sb.append(t_[:, 0:257])
            os_sb.append(t_[:, 257:514])

        def bcast_row(dram, tag, n):
            t_ = cp.tile([128, n], f32, tag=tag)
            dma(out=t_[:], in_=dram[0:1, :].broadcast_to((128, n)))
            return t_

        id_sb = cp.tile([128, 128], f32, tag="id")
        dma(out=id_sb[:], in_=d_id[:])
        id16_sb = cp.tile([128, 128], f16, tag="id16")
        dma(out=id16_sb[:], in_=d_id16[:])

        # remaining inputs (ordered roughly by first use); issue on the
        # gpsimd queue so the sync queue stays clear for the critical path,
        # and gate them behind the first E/O matmul chain so the rfft inputs
        # get full DMA bandwidth at kernel start
        _gated = []

        def gdma(out, in_):
            bi = nc.gpsimd.dma_start(out=out, in_=in_)
            _gated.append(bi)
            return bi
        res_sb = []
        for j in range(NT):
            t_ = cp.tile([128, D], f32, tag=f"res{j}")
            gdma(out=t_[:], in_=d_res[j * 128:(j + 1) * 128, :])
            res_sb.append(t_)

        cs_sb, snn_sb = [], []
        for i in range(4):
            kw = KB[i + 1] - KB[i]
            t_ = cp.tile([128, T], f16, tag=f"cs{i}")
            gdma(out=t_[0:kw, :], in_=d_cs[KB[i]:KB[i + 1], :])
            cs_sb.append(t_)
        for i in range(4):
            kw = KB[i + 1] - KB[i]
            t_ = cp.tile([128, T], f16, tag=f"sn{i}")
            gdma(out=t_[0:kw, :], in_=d_snn[KB[i]:KB[i + 1], :])
            snn_sb.append(t_)

        glinT_sb, gloutT_sb = [], []
        for i in range(ND):
            t_ = cp.tile([128, D], f16, tag=f"gin{i}")
            gdma(out=t_[:], in_=d_glinT[i * 128:(i + 1) * 128, :])
            glinT_sb.append(t_)
        for i in range(ND):
            t_ = cp.tile([128, D], f16, tag=f"got{i}")
            gdma(out=t_[:], in_=d_gloutT[i * 128:(i + 1) * 128, :])
            gloutT_sb.append(t_)

        wgs_sb, lsw2_sb = [], []
        for i in range(ND):
            t_ = cp.tile([128, C], f16, tag=f"wgs{i}")
            gdma(out=t_[:], in_=d_wgs[i * 128:(i + 1) * 128, :])
            wgs_sb.append(t_)
        for i in range(ND):
            t_ = cp.tile([128, C], f16, tag=f"lsw{i}")
            gdma(out=t_[:], in_=d_lsw2[i * 128:(i + 1) * 128, :])
            lsw2_sb.append(t_)

        chc_sb = []   # per ch-tile: cols [alpha, 1-alpha, v0, z0b]
        for m in range(ND):
            t_ = cp.tile([128, 4], f32, tag=f"chc{m}")
            gdma(out=t_[:], in_=d_chc[m * 128:(m + 1) * 128, :])
            chc_sb.append(t_)
        ccc_sb = cp.tile([C, 3], f32, tag="ccc")
        gdma(out=ccc_sb[:], in_=d_ccc[:])

        lvl_sb = []
        for j in range(NT):
            t_ = cp.tile([128, C], f32, tag=f"lvl{j}")
            gdma(out=t_[:], in_=d_level[j * 128:(j + 1) * 128, :])
            lvl_sb.append(t_)

        if has_gob:
            ones_sb = cp.tile([1, D], f16, tag="ones")
            dma(out=ones_sb[:], in_=d_ones[:])
            gob_sb = cp.tile([1, D], f16, tag="gob")
            dma(out=gob_sb[:], in_=d_gob[:])
        if has_bu:
            ones2_sb = cp.tile([1, D], f16, tag="ones2")
            dma(out=ones2_sb[:], in_=d_ones2[:])
            bu_sb = cp.tile([1, C], f16, tag="bu")
            dma(out=bu_sb[:], in_=d_bu[:])
        if has_ffb:
            fb_sb = cp.tile([128, NF], f32, tag="fb")
            for fi in range(NF):
                dma(out=fb_sb[:, fi:fi + 1], in_=d_fb[fi * 128:(fi + 1) * 128, :])

        # growth row 0 is input-independent (v0 @ W + b): DMA straight through
        dma(out=d_out_growth[0:1, :], in_=d_g0[:])

        # level input transpose (PE filler while the rfft waits on DMA)
        lvT = cp.tile([64, T], f16, tag="lvT")
        for j in range(NT):
            ps = pq.tile([128, 512], f32, tag="s")
            nc.tensor.transpose(ps[0:C, 0:128], lvl_sb[j][:, 0:C], id_sb[:])
            nc.scalar.copy(lvT[:, j * 128:(j + 1) * 128], ps[0:C, 0:128])

        # ======= S1: split-DFT (E/O bins 0..256, fp32) + twiddle combine ====
        # ======= S2: top-8 mask per d -> MR/MI [d, k] (f16) =================
        mr_sb, mi_sb = [], []
        for i in range(ND):
            psER = pp.tile([128, 512], f32, tag="m")
            psEI = pp.tile([128, 512], f32, tag="m")
            psOR = pp.tile([128, 512], f32, tag="m")
            psOI = pp.tile([128, 512], f32, tag="m")
            for (ps, src, tab) in ((psER, xe_sb, c512_sb), (psEI, xe_sb, s512_sb),
                                   (psOR, xo_sb, oc_sb), (psOI, xo_sb, os_sb)):
                for kk in range(4):
                    bi = nc.tensor.matmul(
                        ps[:, 0:257], src[kk][:, i * 128:(i + 1) * 128],
                        tab[kk], start=(kk == 0), stop=(kk == 3))
            if i == 0 and _gated:
                from concourse.tile_rust import add_dep_helper
                for g in _gated:
                    add_dep_helper(g.ins, bi.ins,
                                   reason="bulk DMA yields to rfft inputs")
                _gated.clear()

            xr = cp.tile([128, 512], f32, tag="xr0")
            xi = cp.tile([128, 512], f32, tag="xi0")
            amp = cp.tile([128, 512], f32, tag=f"amp{i % 2}")
            rep = cp.tile([128, 512], f32, tag=f"rep{i % 2}")
            TT = nc.vector.tensor_tensor
            # odd tables carry the twiddle; E mirrors by conjugate symmetry.
            # lo bins 1..256 -> cols 0..255; hi bins (reversed) store the
            # NEGATED imag part; snn rows 256+ are negated on host to match.
            # (only one PSUM operand allowed per DVE op: evac the odd pair)
            nc.scalar.copy(amp[:, 0:257], psOR[:, 0:257])
            nc.scalar.copy(rep[:, 0:257], psOI[:, 0:257])
            TT(xr[:, 0:256], psER[:, 1:257], amp[:, 1:257], AL.add)
            TT(xr[:, 256:511], psER[:, 255:0:-1], amp[:, 255:0:-1], AL.subtract)
            TT(xi[:, 0:256], psEI[:, 1:257], rep[:, 1:257], AL.add)
            TT(xi[:, 256:511], psEI[:, 255:0:-1], rep[:, 255:0:-1], AL.subtract)

            # amplitude^2 and top-8 mask
            nc.scalar.activation(amp[:, 0:F], xr[:, 0:F], AF.Square)
            nc.scalar.activation(rep[:, 0:F], xi[:, 0:F], AF.Square)
            TT(amp[:, 0:F], amp[:, 0:F], rep[:, 0:F], AL.add)
            mx8 = sp.tile([128, 8], f32, tag="mx8")
            nc.vector.max(mx8[:], amp[:, 0:F])
            # top-8 selection as a threshold on the 8th-largest amplitude,
            # fused into the masking multiplies
            mr = cp.tile([128, 512], f16, tag=f"mr{i}")
            mi = cp.tile([128, 512], f16, tag=f"mi{i}")
            nc.vector.scalar_tensor_tensor(mr[:, 0:F], amp[:, 0:F],
                                           mx8[:, 7:8], xr[:, 0:F],
                                           AL.is_ge, AL.mult)
            nc.vector.scalar_tensor_tensor(mi[:, 0:F], amp[:, 0:F],
                                           mx8[:, 7:8], xi[:, 0:F],
                                           AL.is_ge, AL.mult)
            mr_sb.append(mr)
            mi_sb.append(mi)

        # ======= S3: MRt/MIt [k,d] (f16) and MRW/MIW [k,c] (f16) =========
        mrt_sb, mit_sb = [], []
        for kk in range(4):
            kw = KB[kk + 1] - KB[kk]
            for (src, dstl, tg) in ((mr_sb, mrt_sb, "mrt"), (mi_sb, mit_sb, "mit")):
                ps = pp.tile([128, 512], f32, tag="m")
                for i in range(ND):
                    nc.tensor.matmul(
                        ps[0:kw, i * 128:(i + 1) * 128],
                        src[i][:, KB[kk]:KB[kk + 1]], id16_sb[:],
                        start=True, stop=True)
                t_ = cp.tile([128, 512], f16, tag=f"{tg}{kk}")
                nc.scalar.copy(t_[0:kw, :], ps[0:kw, :])
                dstl.append(t_)

        mrw_sb, miw_sb = [], []
        for kk in range(4):
            kw = KB[kk + 1] - KB[kk]
            for (src, dstl, tg) in ((mr_sb, mrw_sb, "mrw"), (mi_sb, miw_sb, "miw")):
                ps = pq.tile([128, 512], f32, tag="s")
                for i in range(ND):
                    nc.tensor.matmul(
                        ps[0:kw, 0:C], src[i][:, KB[kk]:KB[kk + 1]],
                        lsw2_sb[i][:], start=(i == 0), stop=(i == ND - 1))
                t_ = cp.tile([128, C], f16, tag=f"{tg}{kk}")
                nc.scalar.copy(t_[0:kw, :], ps[0:kw, 0:C])
                dstl.append(t_)

        # ======= S4: season [tau,d]; res2 = res - season; season out =====
        sea_sb, res2_sb = [], []
        for j in range(NT):
            ps = pp.tile([128, 512], f32, tag="m")
            for kk in range(4):
                kw = KB[kk + 1] - KB[kk]
                nc.tensor.matmul(
                    ps[:], cs_sb[kk][0:kw, j * 128:(j + 1) * 128],
                    mrt_sb[kk][0:kw, 0:D], start=(kk == 0), stop=False)
                nc.tensor.matmul(
                    ps[:], snn_sb[kk][0:kw, j * 128:(j + 1) * 128],
                    mit_sb[kk][0:kw, 0:D], start=False, stop=(kk == 3))
            sea = cp.tile([128, D], f32, tag=f"sea{j % 4}")
            nc.scalar.copy(sea[:], ps[:])
            r2 = cp.tile([128, D], f32, tag=f"r2{j}")
            nc.vector.tensor_tensor(r2[:], res_sb[j][:], ps[:], AL.subtract)
            sea_sb.append(sea)
            res2_sb.append(r2)
            dma(out=d_out_season[j * 128:(j + 1) * 128, :], in_=sea[:])
            if j < 2:   # periodic extension: rows 1024..1279 = rows 0..255
                dma(out=d_out_season[T + j * 128:T + (j + 1) * 128, :], in_=sea[:])

        # ======= S5: res2T [d,t] (f16) ===================================
        res2T_sb = []
        for i in range(ND):
            t_ = cp.tile([128, T], f16, tag=f"r2t{i}")
            for jh in range(2):
                ps = pp.tile([128, 512], f32, tag="m")
                for j4 in range(4):
                    j = jh * 4 + j4
                    nc.tensor.transpose(
                        ps[:, j4 * 128:(j4 + 1) * 128],
                        res2_sb[j][:, i * 128:(i + 1) * 128], id_sb[:])
                nc.scalar.copy(t_[:, jh * 512:(jh + 1) * 512], ps[:])
            res2T_sb.append(t_)

        # ======= S6: vT [ch,t] = glinT.T @ res2T  (into vd slots) ========
        vT_sb = []
        for m in range(ND):
            t_ = cp.tile([128, T], f32, tag=f"vd{m}")
            for th in range(2):
                ps = pp.tile([128, 512], f32, tag="m")
                for i in range(ND):
                    nc.tensor.matmul(
                        ps[:], glinT_sb[i][:, m * 128:(m + 1) * 128],
                        res2T_sb[i][:, th * 512:(th + 1) * 512],
                        start=(i == 0), stop=(i == ND - 1))
                nc.scalar.copy(t_[:, th * 512:(th + 1) * 512], ps[:])
            vT_sb.append(t_)

        # ======= S7: vdiff, u=(1-a)*vd, EMA scan -> sT [ch, 1+t] (f16) ===
        sT_sb = []
        for m in range(ND):
            vd = cp.tile([128, T], f32, tag=f"r2t{m}")
            nc.vector.tensor_tensor(vd[:, 1:T], vT_sb[m][:, 1:T],
                                    vT_sb[m][:, 0:T - 1], AL.subtract)
            nc.vector.tensor_tensor(vd[:, 0:1], vT_sb[m][:, 0:1],
                                    chc_sb[m][:, 3:4], AL.subtract)
            u = vT_sb[m]   # overwrite vT slot elementwise from vd
            nc.vector.tensor_scalar(u[:], vd[:], chc_sb[m][:, 1:2], None, AL.mult)
            st = cp.tile([128, 1056], f16, tag=f"st{m}")
            nc.vector.tensor_copy(st[:, 0:1], chc_sb[m][:, 2:3])
            nc.vector.tensor_tensor_scan(
                st[:, 1:T + 1], chc_sb[m][:, 0:1].broadcast_to((128, T)), u[:],
                chc_sb[m][:, 2:3], AL.mult, AL.add)
            sT_sb.append(st)

        # ======= level path: u-accum [c,t], scan, out ====================
        usb = cp.tile([64, T], f32, tag="r2t0")
        lvs = cp.tile([64, T], f32, tag="r2t1")
        for th in range(2):
            ps = pq.tile([128, 512], f32, tag="s")
            for kk in range(4):
                kw = KB[kk + 1] - KB[kk]
                nc.tensor.matmul(ps[0:C, :], mrw_sb[kk][0:kw, :],
                                 cs_sb[kk][0:kw, th * 512:(th + 1) * 512],
                                 start=(kk == 0), stop=False)
                nc.tensor.matmul(ps[0:C, :], miw_sb[kk][0:kw, :],
                                 snn_sb[kk][0:kw, th * 512:(th + 1) * 512],
                                 start=False, stop=False)
            for m in range(ND):
                nc.tensor.matmul(ps[0:C, :], wgs_sb[m][:],
                                 sT_sb[m][:, th * 512:(th + 1) * 512],
                                 start=False, stop=(m == ND - 1 and not has_bu))
            if has_bu:
                nc.tensor.matmul(ps[0:C, :], bu_sb[:], ones2_sb[:],
                                 start=False, stop=True)
            nc.vector.scalar_tensor_tensor(
                usb[:, th * 512:(th + 1) * 512], lvT[:, th * 512:(th + 1) * 512],
                ccc_sb[:, 1:2], ps[0:C, :], AL.mult, AL.add)
        nc.vector.tensor_tensor_scan(
            lvs[:], ccc_sb[:, 0:1].broadcast_to((64, T)), usb[:],
            ccc_sb[:, 2:3], AL.mult, AL.add)
        for j in range(NT):
            ps = pq.tile([128, 512], f32, tag="s")
            nc.tensor.transpose(ps[:, 0:C], lvs[:, j * 128:(j + 1) * 128],
                                id_sb[0:64, 0:64])
            lo = so.tile([128, C], f32, tag="lvo")
            nc.scalar.copy(lo[:], ps[:, 0:C])
            dma(out=d_out_level[j * 128:(j + 1) * 128, :], in_=lo[:])

        # FF weights arrive into slots freed by the level/synthesis stages
        ffw1_sb = []
        for i in range(ND):
            for h in range(2):
                t_ = cp.tile([128, T], f16, tag=(f"cs{i}" if h == 0 else f"sn{i}"))
                gdma(out=t_[:], in_=d_ffw1T[i * 128:(i + 1) * 128,
                                           h * 1024:(h + 1) * 1024])
                ffw1_sb.append(t_)  # index 2*i + h

        ffw2_sb = []
        for f in range(8):
            t_ = cp.tile([128, 1024], f16, tag=f"ff2{f}")
            gdma(out=t_[:], in_=d_ffw2T[f * 128:(f + 1) * 128, :])
            ffw2_sb.append(t_)

        # ======= S8: growth rows 1..1024; x1 = res2 - growth[1:] =========
        x1_sb = []
        for j in range(NT):
            ps = pp.tile([128, 512], f32, tag="m")
            for m in range(ND):
                nc.tensor.matmul(
                    ps[:], sT_sb[m][:, j * 128 + 1:(j + 1) * 128 + 1],
                    gloutT_sb[m][:], start=(m == 0),
                    stop=(m == ND - 1 and not has_gob))
            if has_gob:
                nc.tensor.matmul(ps[:], ones_sb[0:1, 0:128], gob_sb[:],
                                 start=False, stop=True)
            x1 = cp.tile([128, D], f32, tag=f"sea{j % 4}")
            nc.vector.tensor_tensor(x1[:], res2_sb[j][:], ps[:], AL.subtract)
            gr = cp.tile([128, D], f32, tag=f"r2{j}")
            nc.scalar.copy(gr[:], ps[:])
            dma(out=d_out_growth[j * 128 + 1:(j + 1) * 128 + 1, :], in_=gr[:])
            x1_sb.append(x1)

        # ======= layer norm: z = (x - mean) * rstd (gamma/beta folded) ===
        def norm_z(x_in, out_tile, j):
            st6 = sp.tile([128, 6], f32, tag="st6")
            nc.vector.bn_stats(st6[:], x_in[:])
            mv = sp.tile([128, 2], f32, tag="mv")
            nc.vector.bn_aggr(mv[:], st6[:])
            std = sp.tile([128, 1], f32, tag="col")
            nc.scalar.activation(std[:], mv[:, 1:2], AF.Sqrt, bias=eps_col[:])
            rstd = sp.tile([128, 1], f32, tag="col")
            nc.vector.reciprocal(rstd[:], std[:])
            nc.vector.tensor_scalar(out_tile[:], x_in[:], mv[:, 0:1], rstd[:],
                                    AL.subtract, AL.mult)
            return out_tile

        # ======= S9: norm1 -> z1 [t,d] (gamma1 folded into ffw1) =========
        res3_sb = []
        for j in range(NT):
            out = cp.tile([128, D], f32, tag=f"res{j}")
            norm_z(x1_sb[j], out, j)
            res3_sb.append(out)

        # ======= S10: res3T [d,t] (f16) ==================================
        res3T_sb = []
        for i in range(ND):
            t_ = cp.tile([128, T], f16, tag=f"vd{i}")
            for jh in range(2):
                ps = pp.tile([128, 512], f32, tag="m")
                for j4 in range(4):
                    j = jh * 4 + j4
                    nc.tensor.transpose(
                        ps[:, j4 * 128:(j4 + 1) * 128],
                        res3_sb[j][:, i * 128:(i + 1) * 128], id_sb[:])
                nc.scalar.copy(t_[:, jh * 512:(jh + 1) * 512], ps[:])
            res3T_sb.append(t_)

        # ======= S11: FF1 (gamma1-scaled weights) + sigmoid(+bias) =======
        sig_sb = []
        for fi in range(NF):
            h, fo = fi // 8, fi % 8
            sg = cp.tile([128, T], f16, tag=f"sg{fi}")
            for th in range(2):
                ps = pp.tile([128, 512], f32, tag="m")
                for i in range(ND):
                    nc.tensor.matmul(
                        ps[:], ffw1_sb[2 * i + h][:, fo * 128:(fo + 1) * 128],
                        res3T_sb[i][:, th * 512:(th + 1) * 512],
                        start=(i == 0), stop=(i == ND - 1))
                if has_ffb:
                    nc.scalar.activation(sg[:, th * 512:(th + 1) * 512], ps[:],
                                         AF.Sigmoid, bias=fb_sb[:, fi:fi + 1])
                else:
                    nc.scalar.activation(sg[:, th * 512:(th + 1) * 512], ps[:],
                                         AF.Sigmoid)
            sig_sb.append(sg)

        # ======= S12/S13: FF2 + residual + norm2 -> out ==================
        for j in range(NT):
            ps = pp.tile([128, 512], f32, tag="m")
            for fi in range(NF):
                nc.tensor.matmul(
                    ps[:], sig_sb[fi][:, j * 128:(j + 1) * 128],
                    ffw2_sb[fi % 8][:, (fi // 8) * 512:(fi // 8 + 1) * 512],
                    start=(fi == 0), stop=(fi == NF - 1))
            u2 = cp.tile([128, D], f32, tag=f"sea{j % 4}")
            nc.vector.tensor_tensor(u2[:], res3_sb[j][:], ps[:], AL.add)
            out = cp.tile([128, D], f32, tag=f"st{j % 4}")
            norm_z(u2, out, j)
            dma(out=d_out_res[j * 128:(j + 1) * 128, :], in_=out[:])

    nc.compile()
    return nc


def _host_prep(inputs):
    """Build per-core input maps (numpy only)."""
    def sig(x):
        return 1.0 / (1.0 + np.exp(-x.astype(np.float64)))

    res = np.ascontiguousarray(inputs["res"], dtype=np.float32)
    level = np.ascontiguousarray(inputs["level"], dtype=np.float32)

    tp = np.arange(512)
    k2 = np.arange(257)
    ang_e = 2.0 * np.pi * np.outer(2 * tp, k2) / T
    ang_o = 2.0 * np.pi * np.outer(2 * tp + 1, k2) / T
    cs512 = np.concatenate(
        [np.cos(ang_e), -np.sin(ang_e)], axis=1).astype(np.float32)  # (512, 514)
    os512 = np.concatenate(
        [np.cos(ang_o), -np.sin(ang_o)], axis=1).astype(np.float32)

    t = np.arange(T)
    k = np.arange(1, F + 1)
    ang_kt = 2.0 * np.pi * np.outer(k, t) / T
    cs = (np.cos(ang_kt) * (2.0 / T)).astype(np.float16)
    snn_f = -np.sin(ang_kt) * (2.0 / T)
    snn_f[256:] = -snn_f[256:]   # hi bins store negated imag part on device
    snn = snn_f.astype(np.float16)

    gl_in_w = inputs["gl_in_w"].astype(np.float64)
    gl_out_w = inputs["gl_out_w"].astype(np.float64)
    alpha_ch = np.repeat(sig(inputs["gl_sw"]).reshape(-1), 64)      # (512,)
    v0_ch = inputs["gl_v0"].reshape(-1).astype(np.float64)
    z0b = (inputs["gl_z0"].reshape(-1).astype(np.float64)
           - inputs["gl_in_b"].astype(np.float64))

    alpha_c = sig(inputs["ll_sw"]).reshape(-1)                      # (64,)
    ll_gw = inputs["ll_gw"].astype(np.float64)
    ll_sw2 = inputs["ll_sw2"].astype(np.float64)
    b_g = (inputs["gl_out_b"].astype(np.float64) @ ll_gw.T
           + inputs["ll_gb"].astype(np.float64))
    wgs = (gl_out_w.T @ ll_gw.T) * alpha_c[None, :]
    lsw2 = ll_sw2.T * (-(1.0 - alpha_c))[None, :]
    bias_u = (-(1.0 - alpha_c) * inputs["ll_sb"].astype(np.float64)
              + alpha_c * b_g)

    chc = np.stack([alpha_ch, 1.0 - alpha_ch, v0_ch, z0b], axis=1)
    ccc = np.stack([alpha_c, 1.0 - alpha_c,
                    inputs["ll_v0"].reshape(-1).astype(np.float64)], axis=1)
    g0 = v0_ch @ gl_out_w.T + inputs["gl_out_b"].astype(np.float64)

    # layernorm gamma/beta folding: norm1's gamma is absorbed into ff_w1 and
    # its beta into the sigmoid bias. The residual/norm2 gamma/beta must be
    # identity for this build (true for the reference model: gamma=1, beta=0).
    n1g = inputs["n1_g"].astype(np.float64)
    n1b = inputs["n1_b"].astype(np.float64)
    assert np.all(n1g == 1.0) and np.all(inputs["n2_g"] == 1.0), \
        "non-identity layernorm gamma not supported by this build"
    assert np.all(n1b == 0.0) and np.all(inputs["n2_b"] == 0.0), \
        "nonzero layernorm beta not supported by this build"
    ffw1 = inputs["ff_w1"].astype(np.float64) * n1g[None, :]
    ffb = ffw1 @ n1b   # (FFN,) sigmoid bias

    has_gob = bool(np.any(inputs["gl_out_b"] != 0))
    has_bu = bool(np.any(bias_u != 0))
    has_ffb = bool(np.any(ffb != 0))

    shared = {
        "cs512": cs512, "os512": os512, "cs": cs, "snn": snn,
        "glinT": np.ascontiguousarray(gl_in_w.T, dtype=np.float16),
        "gloutT": np.ascontiguousarray(gl_out_w.T, dtype=np.float16),
        "ffw1T": np.ascontiguousarray(ffw1.T, dtype=np.float16),
                "ffw2T": np.ascontiguousarray(
            np.concatenate([inputs["ff_w2"].T[:FFN // 2],
                            inputs["ff_w2"].T[FFN // 2:]], axis=1),
            dtype=np.float16),
        "wgs": wgs.astype(np.float16),
        "lsw2": lsw2.astype(np.float16),
        "idm": np.eye(128, dtype=np.float32),
        "idm16": np.eye(128, dtype=np.float16),
        "chc": chc.astype(np.float32),
        "ccc": ccc.astype(np.float32),
        "g0": g0.astype(np.float32).reshape(1, D),
    }
    if has_gob:
        shared["onesr"] = np.ones((1, D), dtype=np.float16)
        shared["gob"] = inputs["gl_out_b"].astype(np.float16).reshape(1, D)
    if has_bu:
        shared["onesr2"] = np.ones((1, D), dtype=np.float16)
        shared["bu"] = bias_u.astype(np.float16).reshape(1, C)
    if has_ffb:
        shared["fb"] = ffb.astype(np.float32).reshape(FFN, 1)

    in_maps = []
    for b in range(res.shape[0]):
        m = dict(shared)
        m["res"] = res[b]
        m["level"] = level[b]
        in_maps.append(m)
    return in_maps, (has_gob, has_bu, has_ffb)


def kernel(**inputs):
    _ensure_axon_hooks()
    from concourse.bass_utils import run_bass_kernel_spmd

    in_maps, flags = _host_prep(inputs)
    key = ("nc", flags)
    if key not in _CACHE:
        _CACHE[key] = _build_program(flags)
    nc = _CACHE[key]

    n = len(in_maps)
    kw = {}
    if os.environ.get("KERNEL_TRACE"):
        kw = dict(trace=True, tmpdir=os.environ.get("KERNEL_TRACE_DIR") or None)
    r_ = None
    for attempt in range(3):
        try:
            r_ = run_bass_kernel_spmd(nc, in_maps, list(range(n)), **kw)
            break
        except Exception:
            if attempt == 2:
                raise
            import time
            time.sleep(2.0)
    _CACHE["last_exec_time_ns"] = r_.exec_time_ns

    res_out = np.stack([r_.results[b]["out_res"] for b in range(n)])
    level_out = np.stack([r_.results[b]["out_level"] for b in range(n)])
    growth_out = np.stack([r_.results[b]["out_growth"] for b in range(n)])
    season_out = np.stack([r_.results[b]["out_season"] for b in range(n)])
    return (res_out.astype(np.float32), level_out.astype(np.float32),
            growth_out.astype(np.float32), season_out.astype(np.float32))


# revision 29
# speedup vs baseline: 1.1038x; 1.0331x over previous
"""Trainium2 Bass kernel for nn_EncoderLayer_45423574122725.

Data-parallel over batch: 8 batch elements -> 8 NeuronCores, full pipeline
per core:
  radix-2 split-DFT rfft (fp32 matmuls + DVE twiddle combine) -> top-8 bins
  per (b,d) via DVE max8/match_replace -> masked-spectrum trig resynthesis
  (f16 matmuls) -> growth layer (matmul + first-diff + EMA via
  tensor_tensor_scan) -> layernorm -> sigmoid FF -> layernorm -> level layer
  (fused matmuls + EMA scan).

The FFT smoothing convs in the reference are exact exponential moving
averages (verified algebraically + numerically), implemented with the DVE
tensor_tensor_scan recurrence  state = a*state + b  along the free dim.
"""
import os
import sys
import types

sys.path.insert(0, "/opt/trn_rl_repo")

import numpy as np

import concourse.bacc as bacc
import concourse.bass as bass
import concourse.mybir as mybir
from concourse import tile

f32 = mybir.dt.float32
f16 = mybir.dt.float16
AL = mybir.AluOpType
AF = mybir.ActivationFunctionType
AX = mybir.AxisListType

T = 1024          # seq len
D = 512           # d_model
F = 511           # rfft bins 1..511 (LOW_FREQ=1, Nyquist excluded)
PRED = 256
C = 64            # level channels
FFN = 2048
EPS = 1e-5
NT = T // 128     # 8 time tiles
ND = D // 128     # 4 feature tiles
NF = FFN // 128   # 16 ffn tiles
KB = [0, 128, 256, 384, 511]   # bin-tile boundaries (bin = col+1)

_CACHE: dict = {}


def _ensure_axon_hooks():
    """Install the NTFF profile hook registry if the image's antenv lacks it."""
    try:
        from antenv.axon_hooks import get_axon_ntff_profile_hook  # noqa: F401
        return
    except ImportError:
        pass
    import antenv

    mod = types.ModuleType("antenv.axon_hooks")
    _h = [None]

    def _set(h):
        _h[0] = h

    def _get():
        return _h[0]

    mod.set_axon_ntff_profile_hook = _set
    mod.get_axon_ntff_profile_hook = _get
    sys.modules["antenv.axon_hooks"] = mod
    antenv.axon_hooks = mod
    try:
        from trn_agent_boot.trn_boot import _ntff_profile_via_ctypes
        _set(_ntff_profile_via_ctypes("/opt/axon/libaxon_pjrt.so"))
    except Exception:
        pass


def _build_program(flags):
    """Emit the single-core Bass/Tile program (SPMD across 8 cores).

    flags: (has_gob, has_bu, has_ffb) — whether those bias terms are nonzero.
    """
    has_gob, has_bu, has_ffb = flags
    from concourse import tile_utils
    tile_utils.max_sbuf_usage = 208 * 1024  # cayman usable; default cap is stale

    nc = bacc.Bacc("TRN2", target_bir_lowering=False, debug=False)

    # ---------------- DRAM I/O ----------------
    d_res = nc.dram_tensor("res", [T, D], f32, kind="ExternalInput")
    d_level = nc.dram_tensor("level", [T, C], f32, kind="ExternalInput")
    d_cs512 = nc.dram_tensor("cs512", [512, 514], f32, kind="ExternalInput")
    d_os512 = nc.dram_tensor("os512", [512, 514], f32, kind="ExternalInput")
    d_cs = nc.dram_tensor("cs", [F, T], f16, kind="ExternalInput")        # cos * 2/T
    d_snn = nc.dram_tensor("snn", [F, T], f16, kind="ExternalInput")      # +-sin * 2/T
    d_glinT = nc.dram_tensor("glinT", [D, D], f16, kind="ExternalInput")  # gl_in_w.T
    d_gloutT = nc.dram_tensor("gloutT", [D, D], f16, kind="ExternalInput")
    d_ffw1T = nc.dram_tensor("ffw1T", [D, FFN], f16, kind="ExternalInput")
    d_ffw2T = nc.dram_tensor("ffw2T", [FFN // 2, 2 * D], f16, kind="ExternalInput")
    d_wgs = nc.dram_tensor("wgs", [D, C], f16, kind="ExternalInput")
    d_lsw2 = nc.dram_tensor("lsw2", [D, C], f16, kind="ExternalInput")
    d_id = nc.dram_tensor("idm", [128, 128], f32, kind="ExternalInput")
    d_id16 = nc.dram_tensor("idm16", [128, 128], f16, kind="ExternalInput")
    d_chc = nc.dram_tensor("chc", [D, 4], f32, kind="ExternalInput")      # [a,1-a,v0,z0b]
    d_ccc = nc.dram_tensor("ccc", [C, 3], f32, kind="ExternalInput")      # [a,1-a,v0]
    d_g0 = nc.dram_tensor("g0", [1, D], f32, kind="ExternalInput")        # growth row 0
    if has_gob:
        d_ones = nc.dram_tensor("onesr", [1, D], f16, kind="ExternalInput")
        d_gob = nc.dram_tensor("gob", [1, D], f16, kind="ExternalInput")
    if has_bu:
        d_ones2 = nc.dram_tensor("onesr2", [1, D], f16, kind="ExternalInput")
        d_bu = nc.dram_tensor("bu", [1, C], f16, kind="ExternalInput")
    if has_ffb:
        d_fb = nc.dram_tensor("fb", [FFN, 1], f32, kind="ExternalInput")

    d_out_res = nc.dram_tensor("out_res", [T, D], f32, kind="ExternalOutput")
    d_out_level = nc.dram_tensor("out_level", [T, C], f32, kind="ExternalOutput")
    d_out_growth = nc.dram_tensor("out_growth", [T + 1, D], f32, kind="ExternalOutput")
    d_out_season = nc.dram_tensor("out_season", [T + PRED, D], f32, kind="ExternalOutput")

    from contextlib import ExitStack
    with tile.TileContext(nc) as tc, ExitStack() as _es:
        cp = _es.enter_context(tc.tile_pool(name="cp", bufs=1))
        sp = _es.enter_context(tc.tile_pool(name="sp", bufs=6))
        so = _es.enter_context(tc.tile_pool(name="so", bufs=2))
        pp = _es.enter_context(tc.tile_pool(name="pp", bufs=6, space="PSUM"))
        pq = _es.enter_context(tc.tile_pool(name="pq", bufs=2, space="PSUM"))

        dma = nc.sync.dma_start

        eps_col = cp.tile([128, 1], f32, tag="eps")
        nc.vector.memset(eps_col[:], EPS)

        # ======== stage-0 DMAs (front-of-queue: what the PE needs first) ====
        # even/odd rows of res into the slots later reused by sigmoid tiles
        d_res_eo = d_res.rearrange("(a two) d -> a two d", two=2)
        xe_sb, xo_sb = [], []
        for j in range(4):
            t_ = cp.tile([128, D], f32, tag=f"sg{j}")
            dma(out=t_[:], in_=d_res_eo[j * 128:(j + 1) * 128, 0, :])
            xe_sb.append(t_)
        for j in range(4):
            t_ = cp.tile([128, D], f32, tag=f"sg{4 + j}")
            dma(out=t_[:], in_=d_res_eo[j * 128:(j + 1) * 128, 1, :])
            xo_sb.append(t_)

        c512_sb, s512_sb = [], []
        for kk in range(4):
            t_ = cp.tile([128, 514], f32, tag=f"e5{kk}")
            dma(out=t_[:], in_=d_cs512[kk * 128:(kk + 1) * 128, :])
            c512_sb.append(t_[:, 0:257])
            s512_sb.append(t_[:, 257:514])
        oc_sb, os_sb = [], []
        for kk in range(4):
            t_ = cp.tile([128, 514], f32, tag=f"o5{kk}")
            dma(out=t_[:], in_=d_os512[kk * 128:(kk + 1) * 128, :])
            oc_sb.append(t_[:, 0:257])
            os_sb.append(t_[:, 257:514])

        def bcast_row(dram, tag, n):
            t_ = cp.tile([128, n], f32, tag=tag)
            dma(out=t_[:], in_=dram[0:1, :].broadcast_to((128, n)))
            return t_

        id_sb = cp.tile([128, 128], f32, tag="id")
        dma(out=id_sb[:], in_=d_id[:])
        id16_sb = cp.tile([128, 128], f16, tag="id16")
        dma(out=id16_sb[:], in_=d_id16[:])

        # remaining inputs (ordered roughly by first use); issue on the
        # gpsimd queue so the sync queue stays clear for the critical path,
        # and gate them behind the first E/O matmul chain so the rfft inputs
        # get full DMA bandwidth at kernel start
        _gated = []

        def gdma(out, in_):
            bi = nc.gpsimd.dma_start(out=out, in_=in_)
            _gated.append(bi)
            return bi
        res_sb = []
        for j in range(NT):
            t_ = cp.tile([128, D], f32, tag=f"res{j}")
            gdma(out=t_[:], in_=d_res[j * 128:(j + 1) * 128, :])
            res_sb.append(t_)

        cs_sb, snn_sb = [], []
        for i in range(4):
            kw = KB[i + 1] - KB[i]
            t_ = cp.tile([128, T], f16, tag=f"cs{i}")
            gdma(out=t_[0:kw, :], in_=d_cs[KB[i]:KB[i + 1], :])
            cs_sb.append(t_)
        for i in range(4):
            kw = KB[i + 1] - KB[i]
            t_ = cp.tile([128, T], f16, tag=f"sn{i}")
            gdma(out=t_[0:kw, :], in_=d_snn[KB[i]:KB[i + 1], :])
            snn_sb.append(t_)

        glinT_sb, gloutT_sb = [], []
        for i in range(ND):
            t_ = cp.tile([128, D], f16, tag=f"gin{i}")
            gdma(out=t_[:], in_=d_glinT[i * 128:(i + 1) * 128, :])
            glinT_sb.append(t_)
        for i in range(ND):
            t_ = cp.tile([128, D], f16, tag=f"got{i}")
            gdma(out=t_[:], in_=d_gloutT[i * 128:(i + 1) * 128, :])
            gloutT_sb.append(t_)

        wgs_sb, lsw2_sb = [], []
        for i in range(ND):
            t_ = cp.tile([128, C], f16, tag=f"wgs{i}")
            gdma(out=t_[:], in_=d_wgs[i * 128:(i + 1) * 128, :])
            wgs_sb.append(t_)
        for i in range(ND):
            t_ = cp.tile([128, C], f16, tag=f"lsw{i}")
            gdma(out=t_[:], in_=d_lsw2[i * 128:(i + 1) * 128, :])
            lsw2_sb.append(t_)

        chc_sb = []   # per ch-tile: cols [alpha, 1-alpha, v0, z0b]
        for m in range(ND):
            t_ = cp.tile([128, 4], f32, tag=f"chc{m}")
            gdma(out=t_[:], in_=d_chc[m * 128:(m + 1) * 128, :])
            chc_sb.append(t_)
        ccc_sb = cp.tile([C, 3], f32, tag="ccc")
        gdma(out=ccc_sb[:], in_=d_ccc[:])

        lvl_sb = []
        for j in range(NT):
            t_ = cp.tile([128, C], f32, tag=f"lvl{j}")
            gdma(out=t_[:], in_=d_level[j * 128:(j + 1) * 128, :])
            lvl_sb.append(t_)

        if has_gob:
            ones_sb = cp.tile([1, D], f16, tag="ones")
            dma(out=ones_sb[:], in_=d_ones[:])
            gob_sb = cp.tile([1, D], f16, tag="gob")
            dma(out=gob_sb[:], in_=d_gob[:])
        if has_bu:
            ones2_sb = cp.tile([1, D], f16, tag="ones2")
            dma(out=ones2_sb[:], in_=d_ones2[:])
            bu_sb = cp.tile([1, C], f16, tag="bu")
            dma(out=bu_sb[:], in_=d_bu[:])
        if has_ffb:
            fb_sb = cp.tile([128, NF], f32, tag="fb")
            for fi in range(NF):
                dma(out=fb_sb[:, fi:fi + 1], in_=d_fb[fi * 128:(fi + 1) * 128, :])

        # growth row 0 is input-independent (v0 @ W + b): DMA straight through
        dma(out=d_out_growth[0:1, :], in_=d_g0[:])

        # level input transpose (PE filler while the rfft waits on DMA)
        lvT = cp.tile([64, T], f16, tag="lvT")
        for j in range(NT):
            ps = pq.tile([128, 512], f32, tag="s")
            nc.tensor.transpose(ps[0:C, 0:128], lvl_sb[j][:, 0:C], id_sb[:])
            nc.scalar.copy(lvT[:, j * 128:(j + 1) * 128], ps[0:C, 0:128])

        # ======= S1: split-DFT (E/O bins 0..256, fp32) + twiddle combine ====
        # ======= S2: top-8 mask per d -> MR/MI [d, k] (f16) =================
        mr_sb, mi_sb = [], []
        for i in range(ND):
            psER = pp.tile([128, 512], f32, tag="m")
            psEI = pp.tile([128, 512], f32, tag="m")
            psOR = pp.tile([128, 512], f32, tag="m")
            psOI = pp.tile([128, 512], f32, tag="m")
            for (ps, src, tab) in ((psER, xe_sb, c512_sb), (psEI, xe_sb, s512_sb),
                                   (psOR, xo_sb, oc_sb), (psOI, xo_sb, os_sb)):
                for kk in range(4):
                    bi = nc.tensor.matmul(
                        ps[:, 0:257], src[kk][:, i * 128:(i + 1) * 128],
                        tab[kk], start=(kk == 0), stop=(kk == 3))
            if i == 0 and _gated:
                from concourse.tile_rust import add_dep_helper
                for g in _gated:
                    add_dep_helper(g.ins, bi.ins,
                                   reason="bulk DMA yields to rfft inputs")
                _gated.clear()

            xr = cp.tile([128, 512], f32, tag=f"xr{i % 2}")
            xi = cp.tile([128, 512], f32, tag=f"xi{i % 2}")
            amp = cp.tile([128, 512], f32, tag=f"amp{i % 2}")
            rep = cp.tile([128, 512], f32, tag=f"rep{i % 2}")
            TT = nc.vector.tensor_tensor
            # odd tables carry the twiddle; E mirrors by conjugate symmetry.
            # lo bins 1..256 -> cols 0..255; hi bins (reversed) store the
            # NEGATED imag part; snn rows 256+ are negated on host to match.
            # (only one PSUM operand allowed per DVE op: evac the odd pair)
            nc.scalar.copy(amp[:, 0:257], psOR[:, 0:257])
            nc.scalar.copy(rep[:, 0:257], psOI[:, 0:257])
            TT(xr[:, 0:256], psER[:, 1:257], amp[:, 1:257], AL.add)
            TT(xr[:, 256:511], psER[:, 255:0:-1], amp[:, 255:0:-1], AL.subtract)
            TT(xi[:, 0:256], psEI[:, 1:257], rep[:, 1:257], AL.add)
            TT(xi[:, 256:511], psEI[:, 255:0:-1], rep[:, 255:0:-1], AL.subtract)

            # amplitude^2 and top-8 mask
            nc.scalar.activation(amp[:, 0:F], xr[:, 0:F], AF.Square)
            nc.scalar.activation(rep[:, 0:F], xi[:, 0:F], AF.Square)
            TT(amp[:, 0:F], amp[:, 0:F], rep[:, 0:F], AL.add)
            mx8 = sp.tile([128, 8], f32, tag="mx8")
            nc.vector.max(mx8[:], amp[:, 0:F])
            # top-8 selection as a threshold on the 8th-largest amplitude,
            # fused into the masking multiplies
            mr = cp.tile([128, 512], f16, tag=f"mr{i}")
            mi = cp.tile([128, 512], f16, tag=f"mi{i}")
            nc.vector.scalar_tensor_tensor(mr[:, 0:F], amp[:, 0:F],
                                           mx8[:, 7:8], xr[:, 0:F],
                                           AL.is_ge, AL.mult)
            nc.vector.scalar_tensor_tensor(mi[:, 0:F], amp[:, 0:F],
                                           mx8[:, 7:8], xi[:, 0:F],
                                           AL.is_ge, AL.mult)
            mr_sb.append(mr)
            mi_sb.append(mi)

        # ======= S3: MRt/MIt [k,d] (f16) and MRW/MIW [k,c] (f16) =========
        mrt_sb, mit_sb = [], []
        for kk in range(4):
            kw = KB[kk + 1] - KB[kk]
            for (src, dstl, tg) in ((mr_sb, mrt_sb, "mrt"), (mi_sb, mit_sb, "mit")):
                ps = pp.tile([128, 512], f32, tag="m")
                for i in range(ND):
                    nc.tensor.matmul(
                        ps[0:kw, i * 128:(i + 1) * 128],
                        src[i][:, KB[kk]:KB[kk + 1]], id16_sb[:],
                        start=True, stop=True)
                t_ = cp.tile([128, 512], f16, tag=f"{tg}{kk}")
                nc.scalar.copy(t_[0:kw, :], ps[0:kw, :])
                dstl.append(t_)

        mrw_sb, miw_sb = [], []
        for kk in range(4):
            kw = KB[kk + 1] - KB[kk]
            for (src, dstl, tg) in ((mr_sb, mrw_sb, "mrw"), (mi_sb, miw_sb, "miw")):
                ps = pq.tile([128, 512], f32, tag="s")
                for i in range(ND):
                    nc.tensor.matmul(
                        ps[0:kw, 0:C], src[i][:, KB[kk]:KB[kk + 1]],
                        lsw2_sb[i][:], start=(i == 0), stop=(i == ND - 1))
                t_ = cp.tile([128, C], f16, tag=f"{tg}{kk}")
                nc.scalar.copy(t_[0:kw, :], ps[0:kw, 0:C])
                dstl.append(t_)

        # ======= S4: season [tau,d]; res2 = res - season; season out =====
        sea_sb, res2_sb = [], []
        for j in range(NT):
            ps = pp.tile([128, 512], f32, tag="m")
            for kk in range(4):
                kw = KB[kk + 1] - KB[kk]
                nc.tensor.matmul(
                    ps[:], cs_sb[kk][0:kw, j * 128:(j + 1) * 128],
                    mrt_sb[kk][0:kw, 0:D], start=(kk == 0), stop=False)
                nc.tensor.matmul(
                    ps[:], snn_sb[kk][0:kw, j * 128:(j + 1) * 128],
                    mit_sb[kk][0:kw, 0:D], start=False, stop=(kk == 3))
            sea = cp.tile([128, D], f32, tag=f"sea{j % 4}")
            nc.scalar.copy(sea[:], ps[:])
            r2 = cp.tile([128, D], f32, tag=f"r2{j}")
            nc.vector.tensor_tensor(r2[:], res_sb[j][:], ps[:], AL.subtract)
            sea_sb.append(sea)
            res2_sb.append(r2)
            dma(out=d_out_season[j * 128:(j + 1) * 128, :], in_=sea[:])
            if j < 2:   # periodic extension: rows 1024..1279 = rows 0..255
                dma(out=d_out_season[T + j * 128:T + (j + 1) * 128, :], in_=sea[:])

        # ======= S5: res2T [d,t] (f16) ===================================
        res2T_sb = []
        for i in range(ND):
            t_ = cp.tile([128, T], f16, tag=f"r2t{i}")
            for jh in range(2):
                ps = pp.tile([128, 512], f32, tag="m")
                for j4 in range(4):
                    j = jh * 4 + j4
                    nc.tensor.transpose(
                        ps[:, j4 * 128:(j4 + 1) * 128],
                        res2_sb[j][:, i * 128:(i + 1) * 128], id_sb[:])
                nc.scalar.copy(t_[:, jh * 512:(jh + 1) * 512], ps[:])
            res2T_sb.append(t_)

        # ======= S6: vT [ch,t] = glinT.T @ res2T  (into vd slots) ========
        vT_sb = []
        for m in range(ND):
            t_ = cp.tile([128, T], f32, tag=f"vd{m}")
            for th in range(2):
                ps = pp.tile([128, 512], f32, tag="m")
                for i in range(ND):
                    nc.tensor.matmul(
                        ps[:], glinT_sb[i][:, m * 128:(m + 1) * 128],
                        res2T_sb[i][:, th * 512:(th + 1) * 512],
                        start=(i == 0), stop=(i == ND - 1))
                nc.scalar.copy(t_[:, th * 512:(th + 1) * 512], ps[:])
            vT_sb.append(t_)

        # ======= S7: vdiff, u=(1-a)*vd, EMA scan -> sT [ch, 1+t] (f16) ===
        sT_sb = []
        for m in range(ND):
            vd = cp.tile([128, T], f32, tag=f"r2t{m}")
            nc.vector.tensor_tensor(vd[:, 1:T], vT_sb[m][:, 1:T],
                                    vT_sb[m][:, 0:T - 1], AL.subtract)
            nc.vector.tensor_tensor(vd[:, 0:1], vT_sb[m][:, 0:1],
                                    chc_sb[m][:, 3:4], AL.subtract)
            u = vT_sb[m]   # overwrite vT slot elementwise from vd
            nc.vector.tensor_scalar(u[:], vd[:], chc_sb[m][:, 1:2], None, AL.mult)
            st = cp.tile([128, 1056], f16, tag=f"st{m}")
            nc.vector.tensor_copy(st[:, 0:1], chc_sb[m][:, 2:3])
            nc.vector.tensor_tensor_scan(
                st[:, 1:T + 1], chc_sb[m][:, 0:1].broadcast_to((128, T)), u[:],
                chc_sb[m][:, 2:3], AL.mult, AL.add)
            sT_sb.append(st)

        # ======= level path: u-accum [c,t], scan, out ====================
        usb = cp.tile([64, T], f32, tag="r2t0")
        lvs = cp.tile([64, T], f32, tag="r2t1")
        for th in range(2):
            ps = pq.tile([128, 512], f32, tag="s")
            for kk in range(4):
                kw = KB[kk + 1] - KB[kk]
                nc.tensor.matmul(ps[0:C, :], mrw_sb[kk][0:kw, :],
                                 cs_sb[kk][0:kw, th * 512:(th + 1) * 512],
                                 start=(kk == 0), stop=False)
                nc.tensor.matmul(ps[0:C, :], miw_sb[kk][0:kw, :],
                                 snn_sb[kk][0:kw, th * 512:(th + 1) * 512],
                                 start=False, stop=False)
            for m in range(ND):
                nc.tensor.matmul(ps[0:C, :], wgs_sb[m][:],
                                 sT_sb[m][:, th * 512:(th + 1) * 512],
                                 start=False, stop=(m == ND - 1 and not has_bu))
            if has_bu:
                nc.tensor.matmul(ps[0:C, :], bu_sb[:], ones2_sb[:],
                                 start=False, stop=True)
            nc.vector.scalar_tensor_tensor(
                usb[:, th * 512:(th + 1) * 512], lvT[:, th * 512:(th + 1) * 512],
                ccc_sb[:, 1:2], ps[0:C, :], AL.mult, AL.add)
        nc.vector.tensor_tensor_scan(
            lvs[:], ccc_sb[:, 0:1].broadcast_to((64, T)), usb[:],
            ccc_sb[:, 2:3], AL.mult, AL.add)
        for j in range(NT):
            ps = pq.tile([128, 512], f32, tag="s")
            nc.tensor.transpose(ps[:, 0:C], lvs[:, j * 128:(j + 1) * 128],
                                id_sb[0:64, 0:64])
            lo = so.tile([128, C], f32, tag="lvo")
            nc.scalar.copy(lo[:], ps[:, 0:C])
            dma(out=d_out_level[j * 128:(j + 1) * 128, :], in_=lo[:])

        # FF weights arrive into slots freed by the level/synthesis stages
        ffw1_sb = []
        for i in range(ND):
            for h in range(2):
                t_ = cp.tile([128, T], f16, tag=(f"cs{i}" if h == 0 else f"sn{i}"))
                gdma(out=t_[:], in_=d_ffw1T[i * 128:(i + 1) * 128,
                                           h * 1024:(h + 1) * 1024])
                ffw1_sb.append(t_)  # index 2*i + h

        ffw2_sb = []
        for f in range(8):
            tg = f"e5{f}" if f < 4 else f"o5{f - 4}"
            t_ = cp.tile([128, 1024], f16, tag=tg)
            gdma(out=t_[:], in_=d_ffw2T[f * 128:(f + 1) * 128, :])
            ffw2_sb.append(t_)

        # ======= S8: growth rows 1..1024; x1 = res2 - growth[1:] =========
        x1_sb = []
        for j in range(NT):
            ps = pp.tile([128, 512], f32, tag="m")
            for m in range(ND):
                nc.tensor.matmul(
                    ps[:], sT_sb[m][:, j * 128 + 1:(j + 1) * 128 + 1],
                    gloutT_sb[m][:], start=(m == 0),
                    stop=(m == ND - 1 and not has_gob))
            if has_gob:
                nc.tensor.matmul(ps[:], ones_sb[0:1, 0:128], gob_sb[:],
                                 start=False, stop=True)
            x1 = cp.tile([128, D], f32, tag=f"sea{j % 4}")
            nc.vector.tensor_tensor(x1[:], res2_sb[j][:], ps[:], AL.subtract)
            gr = cp.tile([128, D], f32, tag=f"r2{j}")
            nc.scalar.copy(gr[:], ps[:])
            dma(out=d_out_growth[j * 128 + 1:(j + 1) * 128 + 1, :], in_=gr[:])
            x1_sb.append(x1)

        # ======= layer norm: z = (x - mean) * rstd (gamma/beta folded) ===
        def norm_z(x_in, out_tile, j):
            st6 = sp.tile([128, 6], f32, tag="st6")
            nc.vector.bn_stats(st6[:], x_in[:])
            mv = sp.tile([128, 2], f32, tag="mv")
            nc.vector.bn_aggr(mv[:], st6[:])
            std = sp.tile([128, 1], f32, tag="col")
            nc.scalar.activation(std[:], mv[:, 1:2], AF.Sqrt, bias=eps_col[:])
            rstd = sp.tile([128, 1], f32, tag="col")
            nc.vector.reciprocal(rstd[:], std[:])
            nc.vector.tensor_scalar(out_tile[:], x_in[:], mv[:, 0:1], rstd[:],
                                    AL.subtract, AL.mult)
            return out_tile

        # ======= S9: norm1 -> z1 [t,d] (gamma1 folded into ffw1) =========
        res3_sb = []
        for j in range(NT):
            out = cp.tile([128, D], f32, tag=f"res{j}")
            norm_z(x1_sb[j], out, j)
            res3_sb.append(out)

        # ======= S10: res3T [d,t] (f16) ==================================
        res3T_sb = []
        for i in range(ND):
            t_ = cp.tile([128, T], f16, tag=f"vd{i}")
            for jh in range(2):
                ps = pp.tile([128, 512], f32, tag="m")
                for j4 in range(4):
                    j = jh * 4 + j4
                    nc.tensor.transpose(
                        ps[:, j4 * 128:(j4 + 1) * 128],
                        res3_sb[j][:, i * 128:(i + 1) * 128], id_sb[:])
                nc.scalar.copy(t_[:, jh * 512:(jh + 1) * 512], ps[:])
            res3T_sb.append(t_)

        # ======= S11: FF1 (gamma1-scaled weights) + sigmoid(+bias) =======
        sig_sb = []
        for fi in range(NF):
            h, fo = fi // 8, fi % 8
            sg = cp.tile([128, T], f16, tag=f"sg{fi}")
            for th in range(2):
                ps = pp.tile([128, 512], f32, tag="m")
                for i in range(ND):
                    nc.tensor.matmul(
                        ps[:], ffw1_sb[2 * i + h][:, fo * 128:(fo + 1) * 128],
                        res3T_sb[i][:, th * 512:(th + 1) * 512],
                        start=(i == 0), stop=(i == ND - 1))
                if has_ffb:
                    nc.scalar.activation(sg[:, th * 512:(th + 1) * 512], ps[:],
                                         AF.Sigmoid, bias=fb_sb[:, fi:fi + 1])
                else:
                    nc.scalar.activation(sg[:, th * 512:(th + 1) * 512], ps[:],
                                         AF.Sigmoid)
            sig_sb.append(sg)

        # ======= S12/S13: FF2 + residual + norm2 -> out ==================
        for j in range(NT):
            ps = pp.tile([128, 512], f32, tag="m")
            for fi in range(NF):
                nc.tensor.matmul(
                    ps[:], sig_sb[fi][:, j * 128:(j + 1) * 128],
                    ffw2_sb[fi % 8][:, (fi // 8) * 512:(fi // 8 + 1) * 512],
                    start=(fi == 0), stop=(fi == NF - 1))
            u2 = cp.tile([128, D], f32, tag=f"sea{j % 4}")
            nc.vector.tensor_tensor(u2[:], res3_sb[j][:], ps[:], AL.add)
            out = cp.tile([128, D], f32, tag=f"st{j % 4}")
            norm_z(u2, out, j)
            dma(out=d_out_res[j * 128:(j + 1) * 128, :], in_=out[:])

    nc.compile()
    return nc


def _host_prep(inputs):
    """Build per-core input maps (numpy only)."""
    def sig(x):
        return 1.0 / (1.0 + np.exp(-x.astype(np.float64)))

    res = np.ascontiguousarray(inputs["res"], dtype=np.float32)
    level = np.ascontiguousarray(inputs["level"], dtype=np.float32)

    tp = np.arange(512)
    k2 = np.arange(257)
    ang_e = 2.0 * np.pi * np.outer(2 * tp, k2) / T
    ang_o = 2.0 * np.pi * np.outer(2 * tp + 1, k2) / T
    cs512 = np.concatenate(
        [np.cos(ang_e), -np.sin(ang_e)], axis=1).astype(np.float32)  # (512, 514)
    os512 = np.concatenate(
        [np.cos(ang_o), -np.sin(ang_o)], axis=1).astype(np.float32)

    t = np.arange(T)
    k = np.arange(1, F + 1)
    ang_kt = 2.0 * np.pi * np.outer(k, t) / T
    cs = (np.cos(ang_kt) * (2.0 / T)).astype(np.float16)
    snn_f = -np.sin(ang_kt) * (2.0 / T)
    snn_f[256:] = -snn_f[256:]   # hi bins store negated imag part on device
    snn = snn_f.astype(np.float16)

    gl_in_w = inputs["gl_in_w"].astype(np.float64)
    gl_out_w = inputs["gl_out_w"].astype(np.float64)
    alpha_ch = np.repeat(sig(inputs["gl_sw"]).reshape(-1), 64)      # (512,)
    v0_ch = inputs["gl_v0"].reshape(-1).astype(np.float64)
    z0b = (inputs["gl_z0"].reshape(-1).astype(np.float64)
           - inputs["gl_in_b"].astype(np.float64))

    alpha_c = sig(inputs["ll_sw"]).reshape(-1)                      # (64,)
    ll_gw = inputs["ll_gw"].astype(np.float64)
    ll_sw2 = inputs["ll_sw2"].astype(np.float64)
    b_g = (inputs["gl_out_b"].astype(np.float64) @ ll_gw.T
           + inputs["ll_gb"].astype(np.float64))
    wgs = (gl_out_w.T @ ll_gw.T) * alpha_c[None, :]
    lsw2 = ll_sw2.T * (-(1.0 - alpha_c))[None, :]
    bias_u = (-(1.0 - alpha_c) * inputs["ll_sb"].astype(np.float64)
              + alpha_c * b_g)

    chc = np.stack([alpha_ch, 1.0 - alpha_ch, v0_ch, z0b], axis=1)
    ccc = np.stack([alpha_c, 1.0 - alpha_c,
                    inputs["ll_v0"].reshape(-1).astype(np.float64)], axis=1)
    g0 = v0_ch @ gl_out_w.T + inputs["gl_out_b"].astype(np.float64)

    # layernorm gamma/beta folding: norm1's gamma is absorbed into ff_w1 and
    # its beta into the sigmoid bias. The residual/norm2 gamma/beta must be
    # identity for this build (true for the reference model: gamma=1, beta=0).
    n1g = inputs["n1_g"].astype(np.float64)
    n1b = inputs["n1_b"].astype(np.float64)
    assert np.all(n1g == 1.0) and np.all(inputs["n2_g"] == 1.0), \
        "non-identity layernorm gamma not supported by this build"
    assert np.all(n1b == 0.0) and np.all(inputs["n2_b"] == 0.0), \
        "nonzero layernorm beta not supported by this build"
    ffw1 = inputs["ff_w1"].astype(np.float64) * n1g[None, :]
    ffb = ffw1 @ n1b   # (FFN,) sigmoid bias

    has_gob = bool(np.any(inputs["gl_out_b"] != 0))
    has_bu = bool(np.any(bias_u != 0))
    has_ffb = bool(np.any(ffb != 0))

    shared = {
        "cs512": cs512, "os512": os512, "cs": cs, "snn": snn,
        "glinT": np.ascontiguousarray(gl_in_w.T, dtype=np.float16),
        "gloutT": np.ascontiguousarray(gl_out_w.T, dtype=np.float16),
        "ffw1T": np.ascontiguousarray(ffw1.T, dtype=np.float16),
                "ffw2T": np.ascontiguousarray(
            np.concatenate([inputs["ff_w2"].T[:FFN // 2],
                            inputs["ff_w2"].T[FFN // 2:]], axis=1),
            dtype=np.float16),
        "wgs": wgs.astype(np.float16),
        "lsw2": lsw2.astype(np.float16),
        "idm": np.eye(128, dtype=np.float32),
        "idm16": np.eye(128, dtype=np.float16),
        "chc": chc.astype(np.float32),
        "ccc": ccc.astype(np.float32),
        "g0": g0.astype(np.float32).reshape(1, D),
    }
    if has_gob:
        shared["onesr"] = np.ones((1, D), dtype=np.float16)
        shared["gob"] = inputs["gl_out_b"].astype(np.float16).reshape(1, D)
    if has_bu:
        shared["onesr2"] = np.ones((1, D), dtype=np.float16)
        shared["bu"] = bias_u.astype(np.float16).reshape(1, C)
    if has_ffb:
        shared["fb"] = ffb.astype(np.float32).reshape(FFN, 1)

    in_maps = []
    for b in range(res.shape[0]):
        m = dict(shared)
        m["res"] = res[b]
        m["level"] = level[b]
        in_maps.append(m)
    return in_maps, (has_gob, has_bu, has_ffb)


def kernel(**inputs):
    _ensure_axon_hooks()
    from concourse.bass_utils import run_bass_kernel_spmd

    in_maps, flags = _host_prep(inputs)
    key = ("nc", flags)
    if key not in _CACHE:
        _CACHE[key] = _build_program(flags)
    nc = _CACHE[key]

    n = len(in_maps)
    kw = {}
    if os.environ.get("KERNEL_TRACE"):
        kw = dict(trace=True, tmpdir=os.environ.get("KERNEL_TRACE_DIR") or None)
    r_ = None
    for attempt in range(3):
        try:
            r_ = run_bass_kernel_spmd(nc, in_maps, list(range(n)), **kw)
            break
        except Exception:
            if attempt == 2:
                raise
            import time
            time.sleep(2.0)
    _CACHE["last_exec_time_ns"] = r_.exec_time_ns

    res_out = np.stack([r_.results[b]["out_res"] for b in range(n)])
    level_out = np.stack([r_.results[b]["out_level"] for b in range(n)])
    growth_out = np.stack([r_.results[b]["out_growth"] for b in range(n)])
    season_out = np.stack([r_.results[b]["out_season"] for b in range(n)])
    return (res_out.astype(np.float32), level_out.astype(np.float32),
            growth_out.astype(np.float32), season_out.astype(np.float32))


# revision 30
# speedup vs baseline: 1.1057x; 1.0018x over previous
"""Trainium2 Bass kernel for nn_EncoderLayer_45423574122725.

Data-parallel over batch: 8 batch elements -> 8 NeuronCores, full pipeline
per core:
  radix-2 split-DFT rfft (fp32 matmuls + DVE twiddle combine) -> top-8 bins
  per (b,d) via DVE max8/match_replace -> masked-spectrum trig resynthesis
  (f16 matmuls) -> growth layer (matmul + first-diff + EMA via
  tensor_tensor_scan) -> layernorm -> sigmoid FF -> layernorm -> level layer
  (fused matmuls + EMA scan).

The FFT smoothing convs in the reference are exact exponential moving
averages (verified algebraically + numerically), implemented with the DVE
tensor_tensor_scan recurrence  state = a*state + b  along the free dim.
"""
import os
import sys
import types

sys.path.insert(0, "/opt/trn_rl_repo")

import numpy as np

import concourse.bacc as bacc
import concourse.bass as bass
import concourse.mybir as mybir
from concourse import tile

f32 = mybir.dt.float32
f16 = mybir.dt.float16
AL = mybir.AluOpType
AF = mybir.ActivationFunctionType
AX = mybir.AxisListType

T = 1024          # seq len
D = 512           # d_model
F = 511           # rfft bins 1..511 (LOW_FREQ=1, Nyquist excluded)
PRED = 256
C = 64            # level channels
FFN = 2048
EPS = 1e-5
NT = T // 128     # 8 time tiles
ND = D // 128     # 4 feature tiles
NF = FFN // 128   # 16 ffn tiles
KB = [0, 128, 256, 384, 511]   # bin-tile boundaries (bin = col+1)

_CACHE: dict = {}


def _ensure_axon_hooks():
    """Install the NTFF profile hook registry if the image's antenv lacks it."""
    try:
        from antenv.axon_hooks import get_axon_ntff_profile_hook  # noqa: F401
        return
    except ImportError:
        pass
    import antenv

    mod = types.ModuleType("antenv.axon_hooks")
    _h = [None]

    def _set(h):
        _h[0] = h

    def _get():
        return _h[0]

    mod.set_axon_ntff_profile_hook = _set
    mod.get_axon_ntff_profile_hook = _get
    sys.modules["antenv.axon_hooks"] = mod
    antenv.axon_hooks = mod
    try:
        from trn_agent_boot.trn_boot import _ntff_profile_via_ctypes
        _set(_ntff_profile_via_ctypes("/opt/axon/libaxon_pjrt.so"))
    except Exception:
        pass


def _build_program(flags):
    """Emit the single-core Bass/Tile program (SPMD across 8 cores).

    flags: (has_gob, has_bu, has_ffb) — whether those bias terms are nonzero.
    """
    has_gob, has_bu, has_ffb = flags
    from concourse import tile_utils
    tile_utils.max_sbuf_usage = 208 * 1024  # cayman usable; default cap is stale

    nc = bacc.Bacc("TRN2", target_bir_lowering=False, debug=False)

    # ---------------- DRAM I/O ----------------
    d_res = nc.dram_tensor("res", [T, D], f32, kind="ExternalInput")
    d_level = nc.dram_tensor("level", [T, C], f32, kind="ExternalInput")
    d_cs512 = nc.dram_tensor("cs512", [512, 514], f32, kind="ExternalInput")
    d_os512 = nc.dram_tensor("os512", [512, 514], f32, kind="ExternalInput")
    d_cs = nc.dram_tensor("cs", [F, T], f16, kind="ExternalInput")        # cos * 2/T
    d_snn = nc.dram_tensor("snn", [F, T], f16, kind="ExternalInput")      # +-sin * 2/T
    d_glinT = nc.dram_tensor("glinT", [D, D], f16, kind="ExternalInput")  # gl_in_w.T
    d_gloutT = nc.dram_tensor("gloutT", [D, D], f16, kind="ExternalInput")
    d_ffw1T = nc.dram_tensor("ffw1T", [D, FFN], f16, kind="ExternalInput")
    d_ffw2T = nc.dram_tensor("ffw2T", [FFN // 2, 2 * D], f16, kind="ExternalInput")
    d_wgs = nc.dram_tensor("wgs", [D, C], f16, kind="ExternalInput")
    d_lsw2 = nc.dram_tensor("lsw2", [D, C], f16, kind="ExternalInput")
    d_id = nc.dram_tensor("idm", [128, 128], f32, kind="ExternalInput")
    d_id16 = nc.dram_tensor("idm16", [128, 128], f16, kind="ExternalInput")
    d_chc = nc.dram_tensor("chc", [D, 4], f32, kind="ExternalInput")      # [a,1-a,v0,z0b]
    d_ccc = nc.dram_tensor("ccc", [C, 3], f32, kind="ExternalInput")      # [a,1-a,v0]
    d_g0 = nc.dram_tensor("g0", [1, D], f32, kind="ExternalInput")        # growth row 0
    if has_gob:
        d_ones = nc.dram_tensor("onesr", [1, D], f16, kind="ExternalInput")
        d_gob = nc.dram_tensor("gob", [1, D], f16, kind="ExternalInput")
    if has_bu:
        d_ones2 = nc.dram_tensor("onesr2", [1, D], f16, kind="ExternalInput")
        d_bu = nc.dram_tensor("bu", [1, C], f16, kind="ExternalInput")
    if has_ffb:
        d_fb = nc.dram_tensor("fb", [FFN, 1], f32, kind="ExternalInput")

    d_out_res = nc.dram_tensor("out_res", [T, D], f32, kind="ExternalOutput")
    d_out_level = nc.dram_tensor("out_level", [T, C], f32, kind="ExternalOutput")
    d_out_growth = nc.dram_tensor("out_growth", [T + 1, D], f32, kind="ExternalOutput")
    d_out_season = nc.dram_tensor("out_season", [T + PRED, D], f32, kind="ExternalOutput")

    from contextlib import ExitStack
    with tile.TileContext(nc) as tc, ExitStack() as _es:
        cp = _es.enter_context(tc.tile_pool(name="cp", bufs=1))
        sp = _es.enter_context(tc.tile_pool(name="sp", bufs=12))
        so = _es.enter_context(tc.tile_pool(name="so", bufs=4))
        pp = _es.enter_context(tc.tile_pool(name="pp", bufs=6, space="PSUM"))
        pq = _es.enter_context(tc.tile_pool(name="pq", bufs=2, space="PSUM"))

        dma = nc.sync.dma_start

        eps_col = cp.tile([128, 1], f32, tag="eps")
        nc.vector.memset(eps_col[:], EPS)

        # ======== stage-0 DMAs (front-of-queue: what the PE needs first) ====
        # even/odd rows of res into the slots later reused by sigmoid tiles
        d_res_eo = d_res.rearrange("(a two) d -> a two d", two=2)
        xe_sb, xo_sb = [], []
        c512_sb, s512_sb, oc_sb, os_sb = [], [], [], []
        for kk in range(4):
            t_ = cp.tile([128, D], f32, tag=f"sg{kk}")
            dma(out=t_[:], in_=d_res_eo[kk * 128:(kk + 1) * 128, 0, :])
            xe_sb.append(t_)
            t_ = cp.tile([128, 514], f32, tag=f"e5{kk}")
            dma(out=t_[:], in_=d_cs512[kk * 128:(kk + 1) * 128, :])
            c512_sb.append(t_[:, 0:257])
            s512_sb.append(t_[:, 257:514])
        for kk in range(4):
            t_ = cp.tile([128, D], f32, tag=f"sg{4 + kk}")
            dma(out=t_[:], in_=d_res_eo[kk * 128:(kk + 1) * 128, 1, :])
            xo_sb.append(t_)
            t_ = cp.tile([128, 514], f32, tag=f"o5{kk}")
            dma(out=t_[:], in_=d_os512[kk * 128:(kk + 1) * 128, :])
            oc_sb.append(t_[:, 0:257])
            os_sb.append(t_[:, 257:514])

        def bcast_row(dram, tag, n):
            t_ = cp.tile([128, n], f32, tag=tag)
            dma(out=t_[:], in_=dram[0:1, :].broadcast_to((128, n)))
            return t_

        id_sb = cp.tile([128, 128], f32, tag="id")
        dma(out=id_sb[:], in_=d_id[:])
        id16_sb = cp.tile([128, 128], f16, tag="id16")
        dma(out=id16_sb[:], in_=d_id16[:])

        # remaining inputs (ordered roughly by first use); issue on the
        # gpsimd queue so the sync queue stays clear for the critical path,
        # and gate them behind the first E/O matmul chain so the rfft inputs
        # get full DMA bandwidth at kernel start
        _gated = []

        def gdma(out, in_):
            bi = nc.gpsimd.dma_start(out=out, in_=in_)
            _gated.append(bi)
            return bi
        res_sb = []
        for j in range(NT):
            t_ = cp.tile([128, D], f32, tag=f"res{j}")
            gdma(out=t_[:], in_=d_res[j * 128:(j + 1) * 128, :])
            res_sb.append(t_)

        cs_sb, snn_sb = [], []
        for i in range(4):
            kw = KB[i + 1] - KB[i]
            t_ = cp.tile([128, T], f16, tag=f"cs{i}")
            gdma(out=t_[0:kw, :], in_=d_cs[KB[i]:KB[i + 1], :])
            cs_sb.append(t_)
        for i in range(4):
            kw = KB[i + 1] - KB[i]
            t_ = cp.tile([128, T], f16, tag=f"sn{i}")
            gdma(out=t_[0:kw, :], in_=d_snn[KB[i]:KB[i + 1], :])
            snn_sb.append(t_)

        glinT_sb, gloutT_sb = [], []
        for i in range(ND):
            t_ = cp.tile([128, D], f16, tag=f"gin{i}")
            gdma(out=t_[:], in_=d_glinT[i * 128:(i + 1) * 128, :])
            glinT_sb.append(t_)
        for i in range(ND):
            t_ = cp.tile([128, D], f16, tag=f"got{i}")
            gdma(out=t_[:], in_=d_gloutT[i * 128:(i + 1) * 128, :])
            gloutT_sb.append(t_)

        wgs_sb, lsw2_sb = [], []
        for i in range(ND):
            t_ = cp.tile([128, C], f16, tag=f"wgs{i}")
            gdma(out=t_[:], in_=d_wgs[i * 128:(i + 1) * 128, :])
            wgs_sb.append(t_)
        for i in range(ND):
            t_ = cp.tile([128, C], f16, tag=f"lsw{i}")
            gdma(out=t_[:], in_=d_lsw2[i * 128:(i + 1) * 128, :])
            lsw2_sb.append(t_)

        chc_sb = []   # per ch-tile: cols [alpha, 1-alpha, v0, z0b]
        for m in range(ND):
            t_ = cp.tile([128, 4], f32, tag=f"chc{m}")
            gdma(out=t_[:], in_=d_chc[m * 128:(m + 1) * 128, :])
            chc_sb.append(t_)
        ccc_sb = cp.tile([C, 3], f32, tag="ccc")
        gdma(out=ccc_sb[:], in_=d_ccc[:])

        lvl_sb = []
        for j in range(NT):
            t_ = cp.tile([128, C], f32, tag=f"lvl{j}")
            gdma(out=t_[:], in_=d_level[j * 128:(j + 1) * 128, :])
            lvl_sb.append(t_)

        if has_gob:
            ones_sb = cp.tile([1, D], f16, tag="ones")
            dma(out=ones_sb[:], in_=d_ones[:])
            gob_sb = cp.tile([1, D], f16, tag="gob")
            dma(out=gob_sb[:], in_=d_gob[:])
        if has_bu:
            ones2_sb = cp.tile([1, D], f16, tag="ones2")
            dma(out=ones2_sb[:], in_=d_ones2[:])
            bu_sb = cp.tile([1, C], f16, tag="bu")
            dma(out=bu_sb[:], in_=d_bu[:])
        if has_ffb:
            fb_sb = cp.tile([128, NF], f32, tag="fb")
            for fi in range(NF):
                dma(out=fb_sb[:, fi:fi + 1], in_=d_fb[fi * 128:(fi + 1) * 128, :])

        # growth row 0 is input-independent (v0 @ W + b): DMA straight through
        dma(out=d_out_growth[0:1, :], in_=d_g0[:])

        # level input transpose (PE filler while the rfft waits on DMA)
        lvT = cp.tile([64, T], f16, tag="lvT")
        for j in range(NT):
            ps = pq.tile([128, 512], f32, tag="s")
            nc.tensor.transpose(ps[0:C, 0:128], lvl_sb[j][:, 0:C], id_sb[:])
            nc.scalar.copy(lvT[:, j * 128:(j + 1) * 128], ps[0:C, 0:128])

        # ======= S1: split-DFT (E/O bins 0..256, fp32) + twiddle combine ====
        # ======= S2: top-8 mask per d -> MR/MI [d, k] (f16) =================
        mr_sb, mi_sb = [], []
        for i in range(ND):
            psER = pp.tile([128, 512], f32, tag="m")
            psEI = pp.tile([128, 512], f32, tag="m")
            psOR = pp.tile([128, 512], f32, tag="m")
            psOI = pp.tile([128, 512], f32, tag="m")
            for (ps, src, tab) in ((psER, xe_sb, c512_sb), (psEI, xe_sb, s512_sb),
                                   (psOR, xo_sb, oc_sb), (psOI, xo_sb, os_sb)):
                for kk in range(4):
                    bi = nc.tensor.matmul(
                        ps[:, 0:257], src[kk][:, i * 128:(i + 1) * 128],
                        tab[kk], start=(kk == 0), stop=(kk == 3))
            if i == 0 and _gated:
                from concourse.tile_rust import add_dep_helper
                for g in _gated:
                    add_dep_helper(g.ins, bi.ins,
                                   reason="bulk DMA yields to rfft inputs")
                _gated.clear()

            xr = cp.tile([128, 512], f32, tag=f"xr{i % 2}")
            xi = cp.tile([128, 512], f32, tag=f"xi{i % 2}")
            amp = cp.tile([128, 512], f32, tag=f"amp{i % 2}")
            rep = cp.tile([128, 512], f32, tag=f"rep{i % 2}")
            TT = nc.vector.tensor_tensor
            # odd tables carry the twiddle; E mirrors by conjugate symmetry.
            # lo bins 1..256 -> cols 0..255; hi bins (reversed) store the
            # NEGATED imag part; snn rows 256+ are negated on host to match.
            # (only one PSUM operand allowed per DVE op: evac the odd pair)
            nc.scalar.copy(amp[:, 0:257], psOR[:, 0:257])
            nc.scalar.copy(rep[:, 0:257], psOI[:, 0:257])
            TT(xr[:, 0:256], psER[:, 1:257], amp[:, 1:257], AL.add)
            TT(xr[:, 256:511], psER[:, 255:0:-1], amp[:, 255:0:-1], AL.subtract)
            TT(xi[:, 0:256], psEI[:, 1:257], rep[:, 1:257], AL.add)
            TT(xi[:, 256:511], psEI[:, 255:0:-1], rep[:, 255:0:-1], AL.subtract)

            # amplitude^2 and top-8 mask
            nc.scalar.activation(amp[:, 0:F], xr[:, 0:F], AF.Square)
            nc.scalar.activation(rep[:, 0:F], xi[:, 0:F], AF.Square)
            TT(amp[:, 0:F], amp[:, 0:F], rep[:, 0:F], AL.add)
            mx8 = sp.tile([128, 8], f32, tag="mx8")
            nc.vector.max(mx8[:], amp[:, 0:F])
            # top-8 selection as a threshold on the 8th-largest amplitude,
            # fused into the masking multiplies
            mr = cp.tile([128, 512], f16, tag=f"mr{i}")
            mi = cp.tile([128, 512], f16, tag=f"mi{i}")
            nc.vector.scalar_tensor_tensor(mr[:, 0:F], amp[:, 0:F],
                                           mx8[:, 7:8], xr[:, 0:F],
                                           AL.is_ge, AL.mult)
            nc.vector.scalar_tensor_tensor(mi[:, 0:F], amp[:, 0:F],
                                           mx8[:, 7:8], xi[:, 0:F],
                                           AL.is_ge, AL.mult)
            mr_sb.append(mr)
            mi_sb.append(mi)

        # ======= S3: MRt/MIt [k,d] (f16) and MRW/MIW [k,c] (f16) =========
        mrt_sb, mit_sb = [], []
        for kk in range(4):
            kw = KB[kk + 1] - KB[kk]
            for (src, dstl, tg) in ((mr_sb, mrt_sb, "mrt"), (mi_sb, mit_sb, "mit")):
                ps = pp.tile([128, 512], f32, tag="m")
                for i in range(ND):
                    nc.tensor.matmul(
                        ps[0:kw, i * 128:(i + 1) * 128],
                        src[i][:, KB[kk]:KB[kk + 1]], id16_sb[:],
                        start=True, stop=True)
                t_ = cp.tile([128, 512], f16, tag=f"{tg}{kk}")
                nc.scalar.copy(t_[0:kw, :], ps[0:kw, :])
                dstl.append(t_)

        mrw_sb, miw_sb = [], []
        for kk in range(4):
            kw = KB[kk + 1] - KB[kk]
            for (src, dstl, tg) in ((mr_sb, mrw_sb, "mrw"), (mi_sb, miw_sb, "miw")):
                ps = pq.tile([128, 512], f32, tag="s")
                for i in range(ND):
                    nc.tensor.matmul(
                        ps[0:kw, 0:C], src[i][:, KB[kk]:KB[kk + 1]],
                        lsw2_sb[i][:], start=(i == 0), stop=(i == ND - 1))
                t_ = cp.tile([128, C], f16, tag=f"{tg}{kk}")
                nc.scalar.copy(t_[0:kw, :], ps[0:kw, 0:C])
                dstl.append(t_)

        # ======= S4: season [tau,d]; res2 = res - season; season out =====
        sea_sb, res2_sb = [], []
        for j in range(NT):
            ps = pp.tile([128, 512], f32, tag="m")
            for kk in range(4):
                kw = KB[kk + 1] - KB[kk]
                nc.tensor.matmul(
                    ps[:], cs_sb[kk][0:kw, j * 128:(j + 1) * 128],
                    mrt_sb[kk][0:kw, 0:D], start=(kk == 0), stop=False)
                nc.tensor.matmul(
                    ps[:], snn_sb[kk][0:kw, j * 128:(j + 1) * 128],
                    mit_sb[kk][0:kw, 0:D], start=False, stop=(kk == 3))
            sea = cp.tile([128, D], f32, tag=f"sea{j % 4}")
            nc.scalar.copy(sea[:], ps[:])
            r2 = cp.tile([128, D], f32, tag=f"r2{j}")
            nc.vector.tensor_tensor(r2[:], res_sb[j][:], ps[:], AL.subtract)
            sea_sb.append(sea)
            res2_sb.append(r2)
            dma(out=d_out_season[j * 128:(j + 1) * 128, :], in_=sea[:])
            if j < 2:   # periodic extension: rows 1024..1279 = rows 0..255
                dma(out=d_out_season[T + j * 128:T + (j + 1) * 128, :], in_=sea[:])

        # ======= S5: res2T [d,t] (f16) ===================================
        res2T_sb = []
        for i in range(ND):
            t_ = cp.tile([128, T], f16, tag=f"r2t{i}")
            for jh in range(2):
                ps = pp.tile([128, 512], f32, tag="m")
                for j4 in range(4):
                    j = jh * 4 + j4
                    nc.tensor.transpose(
                        ps[:, j4 * 128:(j4 + 1) * 128],
                        res2_sb[j][:, i * 128:(i + 1) * 128], id_sb[:])
                nc.scalar.copy(t_[:, jh * 512:(jh + 1) * 512], ps[:])
            res2T_sb.append(t_)

        # ======= S6: vT [ch,t] = glinT.T @ res2T  (into vd slots) ========
        vT_sb = []
        for m in range(ND):
            t_ = cp.tile([128, T], f32, tag=f"vd{m}")
            for th in range(2):
                ps = pp.tile([128, 512], f32, tag="m")
                for i in range(ND):
                    nc.tensor.matmul(
                        ps[:], glinT_sb[i][:, m * 128:(m + 1) * 128],
                        res2T_sb[i][:, th * 512:(th + 1) * 512],
                        start=(i == 0), stop=(i == ND - 1))
                nc.scalar.copy(t_[:, th * 512:(th + 1) * 512], ps[:])
            vT_sb.append(t_)

        # ======= S7: vdiff, u=(1-a)*vd, EMA scan -> sT [ch, 1+t] (f16) ===
        sT_sb = []
        for m in range(ND):
            vd = cp.tile([128, T], f32, tag=f"r2t{m}")
            nc.vector.tensor_tensor(vd[:, 1:T], vT_sb[m][:, 1:T],
                                    vT_sb[m][:, 0:T - 1], AL.subtract)
            nc.vector.tensor_tensor(vd[:, 0:1], vT_sb[m][:, 0:1],
                                    chc_sb[m][:, 3:4], AL.subtract)
            u = vT_sb[m]   # overwrite vT slot elementwise from vd
            nc.vector.tensor_scalar(u[:], vd[:], chc_sb[m][:, 1:2], None, AL.mult)
            st = cp.tile([128, 1056], f16, tag=f"st{m}")
            nc.vector.tensor_copy(st[:, 0:1], chc_sb[m][:, 2:3])
            nc.vector.tensor_tensor_scan(
                st[:, 1:T + 1], chc_sb[m][:, 0:1].broadcast_to((128, T)), u[:],
                chc_sb[m][:, 2:3], AL.mult, AL.add)
            sT_sb.append(st)

        # ======= level path: u-accum [c,t], scan, out ====================
        usb = cp.tile([64, T], f32, tag="r2t0")
        lvs = cp.tile([64, T], f32, tag="r2t1")
        for th in range(2):
            ps = pq.tile([128, 512], f32, tag="s")
            for kk in range(4):
                kw = KB[kk + 1] - KB[kk]
                nc.tensor.matmul(ps[0:C, :], mrw_sb[kk][0:kw, :],
                                 cs_sb[kk][0:kw, th * 512:(th + 1) * 512],
                                 start=(kk == 0), stop=False)
                nc.tensor.matmul(ps[0:C, :], miw_sb[kk][0:kw, :],
                                 snn_sb[kk][0:kw, th * 512:(th + 1) * 512],
                                 start=False, stop=False)
            for m in range(ND):
                nc.tensor.matmul(ps[0:C, :], wgs_sb[m][:],
                                 sT_sb[m][:, th * 512:(th + 1) * 512],
                                 start=False, stop=(m == ND - 1 and not has_bu))
            if has_bu:
                nc.tensor.matmul(ps[0:C, :], bu_sb[:], ones2_sb[:],
                                 start=False, stop=True)
            nc.vector.scalar_tensor_tensor(
                usb[:, th * 512:(th + 1) * 512], lvT[:, th * 512:(th + 1) * 512],
                ccc_sb[:, 1:2], ps[0:C, :], AL.mult, AL.add)
        nc.vector.tensor_tensor_scan(
            lvs[:], ccc_sb[:, 0:1].broadcast_to((64, T)), usb[:],
            ccc_sb[:, 2:3], AL.mult, AL.add)
        for j in range(NT):
            ps = pq.tile([128, 512], f32, tag="s")
            nc.tensor.transpose(ps[:, 0:C], lvs[:, j * 128:(j + 1) * 128],
                                id_sb[0:64, 0:64])
            lo = so.tile([128, C], f32, tag="lvo")
            nc.scalar.copy(lo[:], ps[:, 0:C])
            dma(out=d_out_level[j * 128:(j + 1) * 128, :], in_=lo[:])

        # FF weights arrive into slots freed by the level/synthesis stages
        ffw1_sb = []
        for i in range(ND):
            for h in range(2):
                t_ = cp.tile([128, T], f16, tag=(f"cs{i}" if h == 0 else f"sn{i}"))
                gdma(out=t_[:], in_=d_ffw1T[i * 128:(i + 1) * 128,
                                           h * 1024:(h + 1) * 1024])
                ffw1_sb.append(t_)  # index 2*i + h

        ffw2_sb = []
        for f in range(8):
            tg = f"e5{f}" if f < 4 else f"o5{f - 4}"
            t_ = cp.tile([128, 1024], f16, tag=tg)
            gdma(out=t_[:], in_=d_ffw2T[f * 128:(f + 1) * 128, :])
            ffw2_sb.append(t_)

        # ======= S8: growth rows 1..1024; x1 = res2 - growth[1:] =========
        x1_sb = []
        for j in range(NT):
            ps = pp.tile([128, 512], f32, tag="m")
            for m in range(ND):
                nc.tensor.matmul(
                    ps[:], sT_sb[m][:, j * 128 + 1:(j + 1) * 128 + 1],
                    gloutT_sb[m][:], start=(m == 0),
                    stop=(m == ND - 1 and not has_gob))
            if has_gob:
                nc.tensor.matmul(ps[:], ones_sb[0:1, 0:128], gob_sb[:],
                                 start=False, stop=True)
            x1 = cp.tile([128, D], f32, tag=f"sea{j % 4}")
            nc.vector.tensor_tensor(x1[:], res2_sb[j][:], ps[:], AL.subtract)
            gr = cp.tile([128, D], f32, tag=f"r2{j}")
            nc.scalar.copy(gr[:], ps[:])
            dma(out=d_out_growth[j * 128 + 1:(j + 1) * 128 + 1, :], in_=gr[:])
            x1_sb.append(x1)

        # ======= layer norm: z = (x - mean) * rstd (gamma/beta folded) ===
        def norm_z(x_in, out_tile, j):
            st6 = sp.tile([128, 6], f32, tag="st6")
            nc.vector.bn_stats(st6[:], x_in[:])
            mv = sp.tile([128, 2], f32, tag="mv")
            nc.vector.bn_aggr(mv[:], st6[:])
            std = sp.tile([128, 1], f32, tag="col")
            nc.scalar.activation(std[:], mv[:, 1:2], AF.Sqrt, bias=eps_col[:])
            rstd = sp.tile([128, 1], f32, tag="col")
            nc.vector.reciprocal(rstd[:], std[:])
            nc.vector.tensor_scalar(out_tile[:], x_in[:], mv[:, 0:1], rstd[:],
                                    AL.subtract, AL.mult)
            return out_tile

        # ======= S9: norm1 -> z1 [t,d] (gamma1 folded into ffw1) =========
        res3_sb = []
        for j in range(NT):
            out = cp.tile([128, D], f32, tag=f"res{j}")
            norm_z(x1_sb[j], out, j)
            res3_sb.append(out)

        # ======= S10: res3T [d,t] (f16) ==================================
        res3T_sb = []
        for i in range(ND):
            t_ = cp.tile([128, T], f16, tag=f"vd{i}")
            for jh in range(2):
                ps = pp.tile([128, 512], f32, tag="m")
                for j4 in range(4):
                    j = jh * 4 + j4
                    nc.tensor.transpose(
                        ps[:, j4 * 128:(j4 + 1) * 128],
                        res3_sb[j][:, i * 128:(i + 1) * 128], id_sb[:])
                nc.scalar.copy(t_[:, jh * 512:(jh + 1) * 512], ps[:])
            res3T_sb.append(t_)

        # ======= S11: FF1 (gamma1-scaled weights) + sigmoid(+bias) =======
        sig_sb = []
        for fi in range(NF):
            h, fo = fi // 8, fi % 8
            sg = cp.tile([128, T], f16, tag=f"sg{fi}")
            for th in range(2):
                ps = pp.tile([128, 512], f32, tag="m")
                for i in range(ND):
                    nc.tensor.matmul(
                        ps[:], ffw1_sb[2 * i + h][:, fo * 128:(fo + 1) * 128],
                        res3T_sb[i][:, th * 512:(th + 1) * 512],
                        start=(i == 0), stop=(i == ND - 1))
                if has_ffb:
                    nc.scalar.activation(sg[:, th * 512:(th + 1) * 512], ps[:],
                                         AF.Sigmoid, bias=fb_sb[:, fi:fi + 1])
                else:
                    nc.scalar.activation(sg[:, th * 512:(th + 1) * 512], ps[:],
                                         AF.Sigmoid)
            sig_sb.append(sg)

        # ======= S12/S13: FF2 + residual + norm2 -> out ==================
        for j in range(NT):
            ps = pp.tile([128, 512], f32, tag="m")
            for fi in range(NF):
                nc.tensor.matmul(
                    ps[:], sig_sb[fi][:, j * 128:(j + 1) * 128],
                    ffw2_sb[fi % 8][:, (fi // 8) * 512:(fi // 8 + 1) * 512],
                    start=(fi == 0), stop=(fi == NF - 1))
            u2 = cp.tile([128, D], f32, tag=f"sea{j % 4}")
            nc.vector.tensor_tensor(u2[:], res3_sb[j][:], ps[:], AL.add)
            out = cp.tile([128, D], f32, tag=f"st{j % 4}")
            norm_z(u2, out, j)
            dma(out=d_out_res[j * 128:(j + 1) * 128, :], in_=out[:])

    nc.compile()
    return nc


def _host_prep(inputs):
    """Build per-core input maps (numpy only)."""
    def sig(x):
        return 1.0 / (1.0 + np.exp(-x.astype(np.float64)))

    res = np.ascontiguousarray(inputs["res"], dtype=np.float32)
    level = np.ascontiguousarray(inputs["level"], dtype=np.float32)

    tp = np.arange(512)
    k2 = np.arange(257)
    ang_e = 2.0 * np.pi * np.outer(2 * tp, k2) / T
    ang_o = 2.0 * np.pi * np.outer(2 * tp + 1, k2) / T
    cs512 = np.concatenate(
        [np.cos(ang_e), -np.sin(ang_e)], axis=1).astype(np.float32)  # (512, 514)
    os512 = np.concatenate(
        [np.cos(ang_o), -np.sin(ang_o)], axis=1).astype(np.float32)

    t = np.arange(T)
    k = np.arange(1, F + 1)
    ang_kt = 2.0 * np.pi * np.outer(k, t) / T
    cs = (np.cos(ang_kt) * (2.0 / T)).astype(np.float16)
    snn_f = -np.sin(ang_kt) * (2.0 / T)
    snn_f[256:] = -snn_f[256:]   # hi bins store negated imag part on device
    snn = snn_f.astype(np.float16)

    gl_in_w = inputs["gl_in_w"].astype(np.float64)
    gl_out_w = inputs["gl_out_w"].astype(np.float64)
    alpha_ch = np.repeat(sig(inputs["gl_sw"]).reshape(-1), 64)      # (512,)
    v0_ch = inputs["gl_v0"].reshape(-1).astype(np.float64)
    z0b = (inputs["gl_z0"].reshape(-1).astype(np.float64)
           - inputs["gl_in_b"].astype(np.float64))

    alpha_c = sig(inputs["ll_sw"]).reshape(-1)                      # (64,)
    ll_gw = inputs["ll_gw"].astype(np.float64)
    ll_sw2 = inputs["ll_sw2"].astype(np.float64)
    b_g = (inputs["gl_out_b"].astype(np.float64) @ ll_gw.T
           + inputs["ll_gb"].astype(np.float64))
    wgs = (gl_out_w.T @ ll_gw.T) * alpha_c[None, :]
    lsw2 = ll_sw2.T * (-(1.0 - alpha_c))[None, :]
    bias_u = (-(1.0 - alpha_c) * inputs["ll_sb"].astype(np.float64)
              + alpha_c * b_g)

    chc = np.stack([alpha_ch, 1.0 - alpha_ch, v0_ch, z0b], axis=1)
    ccc = np.stack([alpha_c, 1.0 - alpha_c,
                    inputs["ll_v0"].reshape(-1).astype(np.float64)], axis=1)
    g0 = v0_ch @ gl_out_w.T + inputs["gl_out_b"].astype(np.float64)

    # layernorm gamma/beta folding: norm1's gamma is absorbed into ff_w1 and
    # its beta into the sigmoid bias. The residual/norm2 gamma/beta must be
    # identity for this build (true for the reference model: gamma=1, beta=0).
    n1g = inputs["n1_g"].astype(np.float64)
    n1b = inputs["n1_b"].astype(np.float64)
    assert np.all(n1g == 1.0) and np.all(inputs["n2_g"] == 1.0), \
        "non-identity layernorm gamma not supported by this build"
    assert np.all(n1b == 0.0) and np.all(inputs["n2_b"] == 0.0), \
        "nonzero layernorm beta not supported by this build"
    ffw1 = inputs["ff_w1"].astype(np.float64) * n1g[None, :]
    ffb = ffw1 @ n1b   # (FFN,) sigmoid bias

    has_gob = bool(np.any(inputs["gl_out_b"] != 0))
    has_bu = bool(np.any(bias_u != 0))
    has_ffb = bool(np.any(ffb != 0))

    shared = {
        "cs512": cs512, "os512": os512, "cs": cs, "snn": snn,
        "glinT": np.ascontiguousarray(gl_in_w.T, dtype=np.float16),
        "gloutT": np.ascontiguousarray(gl_out_w.T, dtype=np.float16),
        "ffw1T": np.ascontiguousarray(ffw1.T, dtype=np.float16),
                "ffw2T": np.ascontiguousarray(
            np.concatenate([inputs["ff_w2"].T[:FFN // 2],
                            inputs["ff_w2"].T[FFN // 2:]], axis=1),
            dtype=np.float16),
        "wgs": wgs.astype(np.float16),
        "lsw2": lsw2.astype(np.float16),
        "idm": np.eye(128, dtype=np.float32),
        "idm16": np.eye(128, dtype=np.float16),
        "chc": chc.astype(np.float32),
        "ccc": ccc.astype(np.float32),
        "g0": g0.astype(np.float32).reshape(1, D),
    }
    if has_gob:
        shared["onesr"] = np.ones((1, D), dtype=np.float16)
        shared["gob"] = inputs["gl_out_b"].astype(np.float16).reshape(1, D)
    if has_bu:
        shared["onesr2"] = np.ones((1, D), dtype=np.float16)
        shared["bu"] = bias_u.astype(np.float16).reshape(1, C)
    if has_ffb:
        shared["fb"] = ffb.astype(np.float32).reshape(FFN, 1)

    in_maps = []
    for b in range(res.shape[0]):
        m = dict(shared)
        m["res"] = res[b]
        m["level"] = level[b]
        in_maps.append(m)
    return in_maps, (has_gob, has_bu, has_ffb)


def kernel(**inputs):
    _ensure_axon_hooks()
    from concourse.bass_utils import run_bass_kernel_spmd

    in_maps, flags = _host_prep(inputs)
    key = ("nc", flags)
    if key not in _CACHE:
        _CACHE[key] = _build_program(flags)
    nc = _CACHE[key]

    n = len(in_maps)
    kw = {}
    if os.environ.get("KERNEL_TRACE"):
        kw = dict(trace=True, tmpdir=os.environ.get("KERNEL_TRACE_DIR") or None)
    r_ = None
    for attempt in range(3):
        try:
            r_ = run_bass_kernel_spmd(nc, in_maps, list(range(n)), **kw)
            break
        except Exception:
            if attempt == 2:
                raise
            import time
            time.sleep(2.0)
    _CACHE["last_exec_time_ns"] = r_.exec_time_ns

    res_out = np.stack([r_.results[b]["out_res"] for b in range(n)])
    level_out = np.stack([r_.results[b]["out_level"] for b in range(n)])
    growth_out = np.stack([r_.results[b]["out_growth"] for b in range(n)])
    season_out = np.stack([r_.results[b]["out_season"] for b in range(n)])
    return (res_out.astype(np.float32), level_out.astype(np.float32),
            growth_out.astype(np.float32), season_out.astype(np.float32))
